# revision 4
# baseline (speedup 1.0000x reference)
import sys
sys.path.insert(0, "/opt/trn_rl_repo")
import numpy as np

N_ATOMS = 10000
N_SPECIES = 8
N_STRUCT = 8
C = 16
N_BASIS = 8
L_MAX = 3
CUTOFF = 5.0
NCORES = 8
NC_AT = N_ATOMS // NCORES

_prog_cache = {}
PROFILE = False
LAST_PROF = []


def _pack(senders, receivers):
    send = np.asarray(senders).astype(np.int64)
    recv = np.asarray(receivers).astype(np.int64)
    order = np.argsort(recv, kind="stable")
    rs = recv[order]
    ss = send[order]
    deg = np.bincount(recv, minlength=N_ATOMS)
    CNT = None
    for c in (8, 7, 6, 5, 4, 3, 2):
        ok = True
        npd = ((NC_AT + c - 1) // c) * c
        for core in range(NCORES):
            d = np.zeros(npd, np.int64)
            d[:NC_AT] = deg[core * NC_AT:(core + 1) * NC_AT]
            if d.reshape(-1, c).sum(1).max() > 128:
                ok = False
                break
        if ok:
            CNT = c
            break
    assert CNT is not None
    NCH = (NC_AT + CNT - 1) // CNT
    NCHE = NCH + (NCH & 1)
    NPAD = CNT * NCH
    starts = np.zeros(N_ATOMS + 1, np.int64)
    starts[1:] = np.cumsum(deg)
    cores = []
    for core in range(NCORES):
        slot_send = np.zeros((128, NCHE), np.int64)
        mask = np.zeros((128, NCHE, CNT), np.float32)
        eidx = np.zeros((128, NCHE), np.int64)  # global edge id (sorted order)
        valid = np.zeros((128, NCHE), bool)
        for k in range(NCH):
            row = 0
            for j in range(CNT):
                r = core * NC_AT + k * CNT + j
                if r >= (core + 1) * NC_AT:
                    continue
                a, b = starts[r], starts[r + 1]
                n = b - a
                slot_send[row:row + n, k] = ss[a:b]
                eidx[row:row + n, k] = np.arange(a, b)
                mask[row:row + n, k, j] = 1.0
                valid[row:row + n, k] = True
                row += n
            assert row <= 128
        cores.append(dict(slot_send=slot_send, mask=mask, valid=valid))
    return CNT, NCH, NCHE, NPAD, cores


def _build(CNT, NCH, NCHE, NPAD):
    import concourse.bass as bass
    import concourse.bacc as bacc
    import concourse.tile as tile
    from concourse import mybir

    f32 = mybir.dt.float32
    bf16 = mybir.dt.bfloat16
    ALU = mybir.AluOpType
    AF = mybir.ActivationFunctionType

    nc = bacc.Bacc("TRN2", target_bir_lowering=False, debug=False,
                   num_devices=NCORES)
    PP_d = nc.dram_tensor("pp", [128, NCHE, 6], f32, kind="ExternalInput").ap()
    MS_d = nc.dram_tensor("msk", [128, NCHE, CNT], f32, kind="ExternalInput").ap()
    HS_d = nc.dram_tensor("hs", [128, NCHE, 16], f32, kind="ExternalInput").ap()
    S2_d = nc.dram_tensor("s2", [128, 32], f32, kind="ExternalInput").ap()
    W3_d = nc.dram_tensor("w3", [16, 16 * 17], f32, kind="ExternalInput").ap()
    WR_d = nc.dram_tensor("wrb", [128, 8, 16], f32, kind="ExternalInput").ap()
    CE_d = nc.dram_tensor("cemb", [16, NPAD], f32, kind="ExternalInput").ap()
    CW_d = nc.dram_tensor("cw", [1, NPAD], f32, kind="ExternalInput").ap()
    OUTH_d = nc.dram_tensor("outh", [16, NPAD], f32, kind="ExternalOutput").ap()
    OUTE_d = nc.dram_tensor("oute", [1, NPAD], f32, kind="ExternalOutput").ap()

    with tile.TileContext(nc) as tc:
        with tc.tile_pool(name="main", bufs=1) as pool, \
             tc.tile_pool(name="gp", bufs=3) as gpool, \
             tc.tile_pool(name="asp", bufs=3) as apool, \
             tc.tile_pool(name="pa", bufs=2, space="PSUM") as ppa, \
             tc.tile_pool(name="pi", bufs=2, space="PSUM") as ppi, \
             tc.tile_pool(name="ph", bufs=2, space="PSUM") as pph:
            PP = pool.tile([128, NCHE, 6], f32, tag="pp")
            S2 = pool.tile([128, 32], f32, tag="s2")
            W3 = pool.tile([16, 16 * 17], f32, tag="w3")
            WR = pool.tile([128, 8, 16], f32, tag="wr")
            CE = pool.tile([16, NPAD], f32, tag="ce")
            CW = pool.tile([1, NPAD], f32, tag="cw")
            for t, d in ((PP, PP_d), (S2, S2_d), (W3, W3_d), (WR, WR_d),
                         (CE, CE_d), (CW, CW_d)):
                nc.sync.dma_start(t[:], d[:])

            RV = pool.tile([128, NCHE, 3], f32, tag="rv")
            U = pool.tile([128, NCHE, 3], f32, tag="u")
            SC = pool.tile([128, NCHE, 12], f32, tag="sc")
            SH = pool.tile([128, NCHE, 16], f32, tag="sh")
            RR = pool.tile([128, NCHE, 16], f32, tag="rr")
            TM = pool.tile([128, NCHE, 16], f32, tag="tm")
            PT = pool.tile([128, NCHE, 64], bf16, tag="pt")
            INV = pool.tile([16, NPAD, 16], f32, tag="inv")

            def sc(i):
                return SC[:, :, i]

            TT = nc.vector.tensor_tensor
            TS = nc.vector.tensor_scalar

            # geometry
            TT(RV[:], PP[:, :, 3:6], PP[:, :, 0:3], ALU.subtract)
            nc.vector.tensor_mul(U[:], RV[:], RV[:])
            nc.vector.reduce_sum(SC[:, :, 0:1], U[:], mybir.AxisListType.X)
            nc.scalar.activation(sc(1), sc(0), AF.Sqrt)          # r
            nc.vector.tensor_scalar_max(sc(2), sc(1), 1e-6)      # rc
            nc.vector.reciprocal(sc(3), sc(2))                   # rinv
            TT(U[:], RV[:], SC[:, :, 3:4].to_broadcast([128, NCHE, 3]), ALU.mult)
            # fc = 0.5*cos(pi*min(r,5)/5)+0.5 ;  cos(x) = -sin(x - pi/2)
            nc.vector.tensor_scalar_min(sc(6), sc(1), CUTOFF)
            TS(sc(6), sc(6), float(np.pi / CUTOFF), float(-np.pi / 2),
               ALU.mult, ALU.add)
            nc.scalar.activation(sc(7), sc(6), AF.Sin)
            TS(sc(4), sc(7), -0.5, 0.5, ALU.mult, ALU.add)       # fc
            TT(sc(5), sc(4), sc(3), ALU.mult)
            nc.vector.tensor_scalar_mul(sc(5), sc(5),
                                        float(np.sqrt(2.0 / CUTOFF)))  # g
            x, y, z = U[:, :, 0], U[:, :, 1], U[:, :, 2]
            x2, y2, z2, xy, yz, xz = (sc(i) for i in (6, 7, 8, 9, 10, 11))
            nc.vector.tensor_mul(x2, x, x)
            nc.vector.tensor_mul(y2, y, y)
            nc.vector.tensor_mul(z2, z, z)
            nc.vector.tensor_mul(xy, x, y)
            nc.vector.tensor_mul(yz, y, z)
            nc.vector.tensor_mul(xz, x, z)
            d_, t_ = sc(0), sc(1)  # r2, r now dead

            def shm(m):
                return SH[:, :, m]

            nc.vector.memset(shm(0), 0.28209479)
            nc.vector.tensor_scalar_mul(shm(1), y, 0.48860251)
            nc.vector.tensor_scalar_mul(shm(2), z, 0.48860251)
            nc.vector.tensor_scalar_mul(shm(3), x, 0.48860251)
            nc.vector.tensor_scalar_mul(shm(4), xy, 1.09254843)
            nc.vector.tensor_scalar_mul(shm(5), yz, 1.09254843)
            TS(shm(6), z2, 3.0 * 0.31539157, -0.31539157, ALU.mult, ALU.add)
            nc.vector.tensor_scalar_mul(shm(7), xz, 1.09254843)
            TT(d_, x2, y2, ALU.subtract)
            nc.vector.tensor_scalar_mul(shm(8), d_, 0.54627422)
            nc.vector.scalar_tensor_tensor(t_, x2, 3.0, y2, ALU.mult, ALU.subtract)
            TT(t_, t_, y, ALU.mult)
            nc.vector.tensor_scalar_mul(shm(9), t_, 0.59004359)
            TT(t_, xy, z, ALU.mult)
            nc.vector.tensor_scalar_mul(shm(10), t_, 2.89061144)
            TS(t_, z2, 5.0 * 0.45704579, -0.45704579, ALU.mult, ALU.add)
            TT(shm(11), t_, y, ALU.mult)
            TT(shm(13), t_, x, ALU.mult)
            TS(t_, z2, 5.0 * 0.37317633, -3.0 * 0.37317633, ALU.mult, ALU.add)
            TT(shm(12), t_, z, ALU.mult)
            TT(t_, d_, z, ALU.mult)
            nc.vector.tensor_scalar_mul(shm(14), t_, 1.44530572)
            nc.vector.scalar_tensor_tensor(t_, y2, -3.0, x2, ALU.mult, ALU.add)
            TT(t_, t_, x, ALU.mult)
            nc.vector.tensor_scalar_mul(shm(15), t_, 0.59004359)

            # radial RR[e,(l,n)] = sum_b g*sin(b*pi*rc/C) * WR[b,(l,n)]
            # sin(b*theta), theta = pi*rc/5 in [0, ~3.27] via Chebyshev:
            # s1 = -sin(theta-pi); 2cos = -2*sin(theta-pi/2); s_b = 2cos*s_{b-1}-s_{b-2}
            C2, SA, SB, TP = sc(0), sc(1), sc(3), sc(4)
            TS(sc(6), sc(2), float(np.pi / CUTOFF), float(-np.pi),
               ALU.mult, ALU.add)
            nc.scalar.activation(sc(7), sc(6), AF.Sin)
            nc.vector.tensor_scalar_mul(SA, sc(7), -1.0)          # s1
            TS(sc(6), sc(2), float(np.pi / CUTOFF), float(-np.pi / 2),
               ALU.mult, ALU.add)
            nc.scalar.activation(sc(7), sc(6), AF.Sin)
            nc.vector.tensor_scalar_mul(C2, sc(7), -2.0)          # 2cos
            bb = SC[:, :, 7:8].to_broadcast([128, NCHE, 16])
            for b in range(1, N_BASIS + 1):
                if b == 1:
                    cur = SA
                elif b == 2:
                    TT(SB, C2, SA, ALU.mult)
                    cur = SB
                else:
                    TT(TP, C2, SB if b % 2 else SA, ALU.mult)
                    dst = SA if b % 2 else SB
                    TT(dst, TP, SA if b % 2 else SB, ALU.subtract)
                    cur = dst
                TT(sc(7), cur, sc(5), ALU.mult)
                wb = WR[:, b - 1:b, :].to_broadcast([128, NCHE, 16])
                if b == 1:
                    TT(RR[:], bb, wb, ALU.mult)
                else:
                    TT(TM[:], bb, wb, ALU.mult)
                    TT(RR[:], RR[:], TM[:], ALU.add)

            # P[e,(m,n)] = SH[m] * RR[l(m),n]   (bf16 out)
            for l in range(L_MAX + 1):
                nm = 2 * l + 1
                sh_v = SH[:, :, l * l:l * l + nm].unsqueeze(3).to_broadcast(
                    [128, NCHE, nm, 4])
                rr_v = RR[:, :, l * 4:l * 4 + 4].unsqueeze(2).to_broadcast(
                    [128, NCHE, nm, 4])
                pt_v = PT[:, :, l * l * 4:(l * l + nm) * 4].rearrange(
                    "p k (m n) -> p k m n", n=4)
                TT(pt_v, sh_v, rr_v, ALU.mult)

            # per-chunk scatter + CG
            HS = pool.tile([128, NCHE, 16], f32, tag="sh")   # reuse SH slab
            MS = pool.tile([128, NCHE, CNT], f32, tag="rr")  # reuse RR slab
            nc.sync.dma_start(HS[:], HS_d[:])
            nc.sync.dma_start(MS[:], MS_d[:])
            F = CNT * 16
            for kp in range(NCHE // 2):
                pa = ppa.tile([128, F], f32, tag="pa")
                for h in (0, 1):
                    k = 2 * kp + h
                    G = gpool.tile([128, CNT, 16], bf16, tag="g")
                    TT(G[:],
                       HS[:, k:k + 1, :].to_broadcast([128, CNT, 16]),
                       MS[:, k, :].unsqueeze(2).to_broadcast([128, CNT, 16]),
                       ALU.mult)
                    nc.tensor.matmul(pa[64 * h:64 * (h + 1), :],
                                     PT[:, k, :],
                                     G[:].rearrange("p a b -> p (a b)"),
                                     start=True, stop=True)
                As = apool.tile([128, F], f32, tag="as")
                nc.scalar.activation(As[:], pa[:], AF.Square)
                for h in (0, 1):
                    k = 2 * kp + h
                    if k < NCH:
                        pi = ppi.tile([16, F], f32, tag="pi")
                        nc.tensor.matmul(pi[:], S2[:, 16 * h:16 * (h + 1)],
                                         As[:], start=True, stop=True)
                        nc.scalar.copy(
                            INV[0:16, k * CNT:(k + 1) * CNT, :],
                            pi[:].rearrange("p (a b) -> p a b", b=16))

            # W apply: h_pre[o,r] = sum_c sum_ln W3[ln, c*17+o] * INV[ln, r, c]
            OUTS = pool.tile([17, NPAD], f32, tag="pp")  # reuse PP slab
            ET = pool.tile([1, NPAD], f32, tag="et")
            off = 0
            while off < NPAD:
                gsz = min(512, NPAD - off)
                ph = pph.tile([17, 512], f32, tag="ph")
                for c in range(16):
                    nc.tensor.matmul(ph[:, 0:gsz],
                                     W3[:, c * 17:(c + 1) * 17],
                                     INV[:, off:off + gsz, c],
                                     start=(c == 0), stop=(c == 15))
                HT = apool.tile([17, 512], f32, tag="ht")
                nc.scalar.copy(HT[:, 0:gsz], ph[:, 0:gsz])
                nc.vector.tensor_mul(OUTS[0:16, off:off + gsz],
                                     HT[0:16, 0:gsz], CE[:, off:off + gsz])
                # move e row (partition 16) to partition 0 via DMA, then add cw
                nc.sync.dma_start(ET[0:1, off:off + gsz], HT[16:17, 0:gsz])
                off += gsz
            EOUT = pool.tile([1, NPAD], f32, tag="eo")
            TT(EOUT[:], ET[:], CW[:], ALU.add)
            nc.sync.dma_start(OUTH_d[:], OUTS[0:16, :])
            nc.sync.dma_start(OUTE_d[:], EOUT[:])
    nc.compile()
    return nc


def kernel(positions, embed, W_rad, W_inv1, W_inv2, w_out, comp_weights,
           senders, receivers, species, structure_ids):
    from concourse import bass_utils

    positions = np.asarray(positions, np.float32)
    embed = np.asarray(embed, np.float32)
    W_rad = np.asarray(W_rad, np.float32)
    W_inv1 = np.asarray(W_inv1, np.float32)
    W_inv2 = np.asarray(W_inv2, np.float32)
    w_out = np.asarray(w_out, np.float32)
    comp_weights = np.asarray(comp_weights, np.float32)
    senders = np.asarray(senders).astype(np.int64)
    receivers = np.asarray(receivers).astype(np.int64)
    species = np.asarray(species).astype(np.int64)
    structure_ids_np = np.asarray(structure_ids).astype(np.int64)

    CNT, NCH, NCHE, NPAD, cores = _pack(senders, receivers)
    key = (CNT, NCH)
    if key not in _prog_cache:
        _prog_cache[key] = _build(CNT, NCH, NCHE, NPAD)
    nc = _prog_cache[key]

    cemb = embed[species]  # [N,16] gather
    # constant device inputs
    S2 = np.zeros((128, 32), np.float32)
    mi = 0
    for l in range(L_MAX + 1):
        for m in range(2 * l + 1):
            for n in range(4):
                for h in (0, 1):
                    S2[h * 64 + mi * 4 + n, h * 16 + l * 4 + n] = \
                        1.0 / np.sqrt(2.0 * l + 1.0)
            mi += 1
    WRB = np.zeros((8, 16), np.float32)
    for l in range(L_MAX + 1):
        WRB[:, l * 4:(l + 1) * 4] = W_rad[l]  # [8,4]
    WRB = np.broadcast_to(WRB[None], (128, 8, 16)).copy()

    def w3_pack(W, wo):
        Waug = np.concatenate([W, wo[:, None]], 1)  # [256,17]
        W3 = np.zeros((16, 16 * 17), np.float32)
        for l in range(L_MAX + 1):
            for n in range(4):
                for c in range(16):
                    W3[l * 4 + n, c * 17:(c + 1) * 17] = \
                        Waug[(l * 4 + n) * 16 + c]
        return W3

    base_maps = []
    for core in range(NCORES):
        cd = cores[core]
        ss, msk, val = cd["slot_send"], cd["mask"], cd["valid"]
        pp = np.zeros((128, NCHE, 6), np.float32)
        # receiver of slot (row,k): derive from mask
        rloc = msk.argmax(2)  # local j; 0 for invalid
        rglob = core * NC_AT + (np.arange(NCHE)[None, :] * CNT + rloc)
        rglob = np.clip(rglob, 0, N_ATOMS - 1)
        pp[:, :, 0:3] = np.where(val[:, :, None], positions[ss], 0.0)
        pp[:, :, 3:6] = np.where(val[:, :, None], positions[rglob], 0.0)
        at = np.arange(core * NC_AT, core * NC_AT + NPAD)
        atc = np.clip(at, 0, N_ATOMS - 1)
        apad = (at < N_ATOMS)
        cemb_t = np.where(apad[None, :], cemb[atc].T, 0.0).astype(np.float32)
        cw_t = np.where(apad, comp_weights[species[atc]], 0.0
                        ).astype(np.float32)[None, :]
        base_maps.append(dict(pp=pp, msk=msk, s2=S2, wrb=WRB,
                              cemb=np.ascontiguousarray(cemb_t),
                              cw=np.ascontiguousarray(cw_t)))

    # launch 1
    hs0 = cemb  # h = cemb initially
    w3_1 = w3_pack(W_inv1, np.zeros(256, np.float32))
    maps1 = []
    for core in range(NCORES):
        cd = cores[core]
        hsl = np.where(cd["valid"][:, :, None],
                       hs0[cd["slot_send"]], 0.0).astype(np.float32)
        maps1.append(dict(base_maps[core], hs=hsl, w3=w3_1))
    res1 = bass_utils.run_bass_kernel_spmd(nc, maps1,
                                           core_ids=list(range(NCORES)),
                                           trace=PROFILE)
    if PROFILE:
        LAST_PROF.append(res1)
    h1 = np.concatenate(
        [res1.results[c]["outh"][:, 0:NC_AT].T for c in range(NCORES)], 0)

    # launch 2
    w3_2 = w3_pack(W_inv2, w_out)
    maps2 = []
    for core in range(NCORES):
        cd = cores[core]
        hsl = np.where(cd["valid"][:, :, None],
                       h1[cd["slot_send"]], 0.0).astype(np.float32)
        maps2.append(dict(base_maps[core], hs=hsl, w3=w3_2))
    res2 = bass_utils.run_bass_kernel_spmd(nc, maps2,
                                           core_ids=list(range(NCORES)),
                                           trace=PROFILE)
    if PROFILE:
        LAST_PROF.append(res2)
    e_atom = np.concatenate(
        [res2.results[c]["oute"][0, 0:NC_AT] for c in range(NCORES)], 0)

    out = np.zeros(N_STRUCT, np.float32)
    np.add.at(out, structure_ids_np, e_atom)
    return out



# revision 5
# speedup vs baseline: 2.0371x; 2.0371x over previous
import sys
sys.path.insert(0, "/opt/trn_rl_repo")
import numpy as np

N_ATOMS = 10000
N_SPECIES = 8
N_STRUCT = 8
C = 16
N_BASIS = 8
L_MAX = 3
CUTOFF = 5.0
NCORES = 8
NC_AT = N_ATOMS // NCORES
CNT_MAX = 6
JC = CNT_MAX * C  # 96

_prog_cache = {}
PROFILE = False
LAST_PROF = []

# mn row order within a 64-row block: (l, m, n), n fastest
_LOF = np.repeat(np.arange(4), [(2 * l + 1) * 4 for l in range(4)])
_MOF = np.concatenate([np.repeat(np.arange(2 * l + 1), 4) for l in range(4)])
_NOF = np.concatenate([np.tile(np.arange(4), 2 * l + 1) for l in range(4)])
_SFAC = np.repeat([1.0 / np.sqrt(2.0 * l + 1.0) for l in range(4)],
                  [(2 * l + 1) * 4 for l in range(4)]).astype(np.float32)


def _pack(senders, receivers):
    """FFD pack receiver atoms into pairs (2 blocks of <=CNT_MAX atoms,
    <=128 edges per pair)."""
    recv = np.asarray(receivers).astype(np.int64)
    send = np.asarray(senders).astype(np.int64)
    order = np.argsort(recv, kind="stable")
    ss = send[order]
    deg = np.bincount(recv, minlength=N_ATOMS)
    starts = np.zeros(N_ATOMS + 1, np.int64)
    starts[1:] = np.cumsum(deg)
    core_pairs = []
    for core in range(NCORES):
        a0 = core * NC_AT
        atoms = sorted(range(a0, a0 + NC_AT), key=lambda a: -deg[a])
        pairs = []  # [edge_count, block0 atoms, block1 atoms]
        for a in atoms:
            placed = False
            for p in pairs:
                if p[0] + deg[a] <= 128:
                    if len(p[1]) < CNT_MAX:
                        p[1].append(a); p[0] += deg[a]; placed = True; break
                    elif len(p[2]) < CNT_MAX:
                        p[2].append(a); p[0] += deg[a]; placed = True; break
            if not placed:
                pairs.append([deg[a], [a], []])
        core_pairs.append(pairs)
    KP = max(len(p) for p in core_pairs)
    if KP % 10:
        KP += 10 - KP % 10  # multiple of 5 (psum groups) and 2 (halves)
    tabs = []
    for core in range(NCORES):
        pairs = core_pairs[core]
        slot_send = np.zeros((128, KP), np.int64)
        slot_val = np.zeros((128, KP), bool)
        MS = np.zeros((128, KP, CNT_MAX), np.float16)
        M2 = np.zeros((128, KP, 2), np.float16)
        amap = np.full((2, KP, CNT_MAX), -1, np.int64)
        for kp, pr in enumerate(pairs):
            row = 0
            for b in range(2):
                for j, a in enumerate(pr[1 + b]):
                    s0, s1 = starts[a], starts[a + 1]
                    n = s1 - s0
                    slot_send[row:row + n, kp] = ss[s0:s1]
                    slot_val[row:row + n, kp] = True
                    MS[row:row + n, kp, j] = 1.0
                    M2[row:row + n, kp, b] = 1.0
                    amap[b, kp, j] = a
                    row += n
        tabs.append(dict(slot_send=slot_send, slot_val=slot_val,
                         MS=MS, M2=M2, amap=amap))
    return KP, tabs


def _build_A(KP):
    import concourse.bass as bass
    import concourse.bacc as bacc
    import concourse.tile as tile
    from concourse import mybir

    f32 = mybir.dt.float32
    f16 = mybir.dt.float16
    ALU = mybir.AluOpType
    AF = mybir.ActivationFunctionType
    KPC = KP * CNT_MAX

    nc = bacc.Bacc("TRN2", target_bir_lowering=False, debug=False,
                   num_devices=NCORES)
    PP_d = nc.dram_tensor("pp", [128, KP, 6], f32, kind="ExternalInput").ap()
    WR_d = nc.dram_tensor("wrb", [128, 8, 16], f32, kind="ExternalInput").ap()
    HS_d = nc.dram_tensor("hs", [128, KP, 16], f16, kind="ExternalInput").ap()
    MS_d = nc.dram_tensor("msk", [128, KP, CNT_MAX], f16,
                          kind="ExternalInput").ap()
    M2_d = nc.dram_tensor("m2", [128, KP, 2], f16, kind="ExternalInput").ap()
    SW_d = nc.dram_tensor("sw", [128, 16, 32], f16, kind="ExternalInput").ap()
    CE_d = nc.dram_tensor("ce", [32, KPC], f32, kind="ExternalInput").ap()
    PT0_d = nc.dram_tensor("pt0", [128, KP, 64], f16,
                           kind="ExternalOutput").ap()
    OUTH_d = nc.dram_tensor("outh", [32, KPC], f32,
                            kind="ExternalOutput").ap()

    with tile.TileContext(nc) as tc:
        with tc.tile_pool(name="main", bufs=1) as pool, \
             tc.tile_pool(name="pa", bufs=3, space="PSUM") as ppa, \
             tc.tile_pool(name="ph", bufs=2, space="PSUM") as pph:
            PP = pool.tile([128, KP, 6], f32, tag="pp")
            WR = pool.tile([128, 8, 16], f32, tag="wr")
            HS = pool.tile([128, KP, 16], f16, tag="hs")
            MS = pool.tile([128, KP, CNT_MAX], f16, tag="ms")
            M2 = pool.tile([128, KP, 2], f16, tag="m2")
            SW = pool.tile([128, 16, 32], f16, tag="sw")
            CE = pool.tile([32, KPC], f32, tag="ce")
            for t, dd in ((PP, PP_d), (WR, WR_d), (HS, HS_d), (MS, MS_d),
                          (M2, M2_d), (SW, SW_d), (CE, CE_d)):
                nc.sync.dma_start(t[:], dd[:])

            RV = pool.tile([128, KP, 3], f32, tag="rv")
            U = pool.tile([128, KP, 3], f32, tag="u")
            SC = pool.tile([128, KP, 12], f32, tag="sc")
            SH = pool.tile([128, KP, 16], f32, tag="sh")
            RR = pool.tile([128, KP, 16], f32, tag="rr")
            TM = pool.tile([128, KP, 16], f32, tag="tm")
            PT0 = pool.tile([128, KP, 64], f16, tag="pt0")
            PT = pool.tile([128, KP, 128], f16, tag="pt")
            G = pool.tile([128, KP, JC], f16, tag="g")
            AS = pool.tile([128, KP, JC], f16, tag="as")
            OUTH = pool.tile([32, KPC], f32, tag="oh")

            def sc(i):
                return SC[:, :, i]

            TT = nc.vector.tensor_tensor
            TS = nc.vector.tensor_scalar

            # geometry
            TT(RV[:], PP[:, :, 3:6], PP[:, :, 0:3], ALU.subtract)
            nc.vector.tensor_mul(U[:], RV[:], RV[:])
            nc.vector.reduce_sum(SC[:, :, 0:1], U[:], mybir.AxisListType.X)
            nc.scalar.activation(sc(1), sc(0), AF.Sqrt)          # r
            nc.vector.tensor_scalar_max(sc(2), sc(1), 1e-6)      # rc
            nc.vector.reciprocal(sc(3), sc(2))                   # rinv
            TT(U[:], RV[:], SC[:, :, 3:4].to_broadcast([128, KP, 3]), ALU.mult)
            # fc = 0.5*cos(pi*min(r,5)/5)+0.5 ;  cos(x) = -sin(x - pi/2)
            nc.vector.tensor_scalar_min(sc(6), sc(1), CUTOFF)
            TS(sc(6), sc(6), float(np.pi / CUTOFF), float(-np.pi / 2),
               ALU.mult, ALU.add)
            nc.scalar.activation(sc(7), sc(6), AF.Sin)
            TS(sc(4), sc(7), -0.5, 0.5, ALU.mult, ALU.add)       # fc
            TT(sc(5), sc(4), sc(3), ALU.mult)
            nc.vector.tensor_scalar_mul(sc(5), sc(5),
                                        float(np.sqrt(2.0 / CUTOFF)))  # g
            x, y, z = U[:, :, 0], U[:, :, 1], U[:, :, 2]
            x2, y2, z2, xy, yz, xz = (sc(i) for i in (6, 7, 8, 9, 10, 11))
            nc.vector.tensor_mul(x2, x, x)
            nc.vector.tensor_mul(y2, y, y)
            nc.vector.tensor_mul(z2, z, z)
            nc.vector.tensor_mul(xy, x, y)
            nc.vector.tensor_mul(yz, y, z)
            nc.vector.tensor_mul(xz, x, z)
            d_, t_ = sc(0), sc(1)  # r2, r now dead

            def shm(m):
                return SH[:, :, m]

            nc.vector.memset(shm(0), 0.28209479)
            nc.vector.tensor_scalar_mul(shm(1), y, 0.48860251)
            nc.vector.tensor_scalar_mul(shm(2), z, 0.48860251)
            nc.vector.tensor_scalar_mul(shm(3), x, 0.48860251)
            nc.vector.tensor_scalar_mul(shm(4), xy, 1.09254843)
            nc.vector.tensor_scalar_mul(shm(5), yz, 1.09254843)
            TS(shm(6), z2, 3.0 * 0.31539157, -0.31539157, ALU.mult, ALU.add)
            nc.vector.tensor_scalar_mul(shm(7), xz, 1.09254843)
            TT(d_, x2, y2, ALU.subtract)
            nc.vector.tensor_scalar_mul(shm(8), d_, 0.54627422)
            nc.vector.scalar_tensor_tensor(t_, x2, 3.0, y2, ALU.mult,
                                           ALU.subtract)
            TT(t_, t_, y, ALU.mult)
            nc.vector.tensor_scalar_mul(shm(9), t_, 0.59004359)
            TT(t_, xy, z, ALU.mult)
            nc.vector.tensor_scalar_mul(shm(10), t_, 2.89061144)
            TS(t_, z2, 5.0 * 0.45704579, -0.45704579, ALU.mult, ALU.add)
            TT(shm(11), t_, y, ALU.mult)
            TT(shm(13), t_, x, ALU.mult)
            TS(t_, z2, 5.0 * 0.37317633, -3.0 * 0.37317633, ALU.mult, ALU.add)
            TT(shm(12), t_, z, ALU.mult)
            TT(t_, d_, z, ALU.mult)
            nc.vector.tensor_scalar_mul(shm(14), t_, 1.44530572)
            nc.vector.scalar_tensor_tensor(t_, y2, -3.0, x2, ALU.mult,
                                           ALU.add)
            TT(t_, t_, x, ALU.mult)
            nc.vector.tensor_scalar_mul(shm(15), t_, 0.59004359)

            # radial RR[e,(l,n)] = sum_b g*sin(b*pi*rc/C) * WR[b,(l,n)]
            # sin(b*theta) via Chebyshev recurrence:
            # s1 = -sin(theta-pi); 2cos = -2*sin(theta-pi/2);
            # s_b = 2cos*s_{b-1}-s_{b-2}
            C2, SA, SB, TP = sc(0), sc(1), sc(3), sc(4)
            TS(sc(6), sc(2), float(np.pi / CUTOFF), float(-np.pi),
               ALU.mult, ALU.add)
            nc.scalar.activation(sc(7), sc(6), AF.Sin)
            nc.vector.tensor_scalar_mul(SA, sc(7), -1.0)          # s1
            TS(sc(6), sc(2), float(np.pi / CUTOFF), float(-np.pi / 2),
               ALU.mult, ALU.add)
            nc.scalar.activation(sc(7), sc(6), AF.Sin)
            nc.vector.tensor_scalar_mul(C2, sc(7), -2.0)          # 2cos
            bb = SC[:, :, 7:8].to_broadcast([128, KP, 16])
            for b in range(1, N_BASIS + 1):
                if b == 1:
                    cur = SA
                elif b == 2:
                    TT(SB, C2, SA, ALU.mult)
                    cur = SB
                else:
                    TT(TP, C2, SB if b % 2 else SA, ALU.mult)
                    dst = SA if b % 2 else SB
                    TT(dst, TP, SA if b % 2 else SB, ALU.subtract)
                    cur = dst
                TT(sc(7), cur, sc(5), ALU.mult)
                wb = WR[:, b - 1:b, :].to_broadcast([128, KP, 16])
                if b == 1:
                    TT(RR[:], bb, wb, ALU.mult)
                else:
                    TT(TM[:], bb, wb, ALU.mult)
                    TT(RR[:], RR[:], TM[:], ALU.add)

            # PT0[e,(m,n)] = SH[m] * RR[l(m),n]   (f16)
            for l in range(L_MAX + 1):
                nm = 2 * l + 1
                o4 = (l * l) * 4
                sh_v = SH[:, :, l * l:l * l + nm].unsqueeze(3).to_broadcast(
                    [128, KP, nm, 4])
                rr_v = RR[:, :, l * 4:l * 4 + 4].unsqueeze(2).to_broadcast(
                    [128, KP, nm, 4])
                pt_v = PT0[:, :, o4:o4 + nm * 4].rearrange(
                    "p k (m n) -> p k m n", n=4)
                TT(pt_v, sh_v, rr_v, ALU.mult)
            nc.sync.dma_start(PT0_d[:], PT0[:])

            # PT[e, (b, mn)] = PT0[e, mn] * M2[e, b]  (block-diag expand)
            pt_b = PT[:].rearrange("p k (b m) -> p k b m", b=2)
            TT(pt_b,
               PT0[:].unsqueeze(2).to_broadcast([128, KP, 2, 64]),
               M2[:].unsqueeze(3).to_broadcast([128, KP, 2, 64]),
               ALU.mult)

            # G[e, (j, c)] = HS[e, c] * MS[e, j]
            g_v = G[:].rearrange("p k (j c) -> p k j c", c=16)
            TT(g_v,
               HS[:].unsqueeze(2).to_broadcast([128, KP, CNT_MAX, 16]),
               MS[:].unsqueeze(3).to_broadcast([128, KP, CNT_MAX, 16]),
               ALU.mult)

            # scatter matmuls: A[(b,mn), (j,c)] per pair; 5 pairs per psum
            for g5 in range(KP // 5):
                pa = ppa.tile([128, 5 * JC], f32, tag="pa")
                for q in range(5):
                    kp = 5 * g5 + q
                    nc.tensor.matmul(pa[:, q * JC:(q + 1) * JC],
                                     PT[:, kp, :], G[:, kp, :],
                                     start=True, stop=True)
                nc.scalar.activation(
                    AS[:, 5 * g5:5 * g5 + 5, :],
                    pa[:].rearrange("p (k f) -> p k f", f=JC),
                    AF.Square)

            # output stage: h1_pre[(b,o), (kp,j)] =
            #   sum_c sum_(b,mn) SW[(b,mn),(b,o)] * AS[(b,mn),(kp,j,c)]
            H = KP // 2
            as_v = AS[:].rearrange("p k (j c) -> p k j c", c=16)
            for grp in range(2):
                ph = pph.tile([32, H * CNT_MAX], f32, tag="ph")
                for c in range(16):
                    nc.tensor.matmul(
                        ph[:],
                        SW[:, c, :],
                        as_v[:, grp * H:(grp + 1) * H, :, c],
                        start=(c == 0), stop=(c == 15))
                cs = grp * H * CNT_MAX
                TT(OUTH[:, cs:cs + H * CNT_MAX], ph[:],
                   CE[:, cs:cs + H * CNT_MAX], ALU.mult)
            nc.sync.dma_start(OUTH_d[:], OUTH[:])
    nc.compile()
    return nc


def _build_B(KP):
    import concourse.bass as bass
    import concourse.bacc as bacc
    import concourse.tile as tile
    from concourse import mybir

    f32 = mybir.dt.float32
    f16 = mybir.dt.float16
    ALU = mybir.AluOpType
    AF = mybir.ActivationFunctionType
    KPC = KP * CNT_MAX

    nc = bacc.Bacc("TRN2", target_bir_lowering=False, debug=False,
                   num_devices=NCORES)
    PT0_d = nc.dram_tensor("pt0", [128, KP, 64], f16,
                           kind="ExternalInput").ap()
    HS_d = nc.dram_tensor("hs", [128, KP, 16], f16, kind="ExternalInput").ap()
    MS_d = nc.dram_tensor("msk", [128, KP, CNT_MAX], f16,
                          kind="ExternalInput").ap()
    M2_d = nc.dram_tensor("m2", [128, KP, 2], f16, kind="ExternalInput").ap()
    SE_d = nc.dram_tensor("se", [128, 16, 2], f16, kind="ExternalInput").ap()
    OUTE_d = nc.dram_tensor("oute", [2, KPC], f32,
                            kind="ExternalOutput").ap()

    with tile.TileContext(nc) as tc:
        with tc.tile_pool(name="main", bufs=1) as pool, \
             tc.tile_pool(name="pa", bufs=3, space="PSUM") as ppa, \
             tc.tile_pool(name="ph", bufs=2, space="PSUM") as pph:
            PT0 = pool.tile([128, KP, 64], f16, tag="pt0")
            HS = pool.tile([128, KP, 16], f16, tag="hs")
            MS = pool.tile([128, KP, CNT_MAX], f16, tag="ms")
            M2 = pool.tile([128, KP, 2], f16, tag="m2")
            SE = pool.tile([128, 16, 2], f16, tag="se")
            for t, dd in ((PT0, PT0_d), (HS, HS_d), (MS, MS_d), (M2, M2_d),
                          (SE, SE_d)):
                nc.sync.dma_start(t[:], dd[:])
            PT = pool.tile([128, KP, 128], f16, tag="pt")
            G = pool.tile([128, KP, JC], f16, tag="g")
            AS = pool.tile([128, KP, JC], f16, tag="as")
            OUTE = pool.tile([2, KPC], f32, tag="oe")

            TT = nc.vector.tensor_tensor
            pt_b = PT[:].rearrange("p k (b m) -> p k b m", b=2)
            TT(pt_b,
               PT0[:].unsqueeze(2).to_broadcast([128, KP, 2, 64]),
               M2[:].unsqueeze(3).to_broadcast([128, KP, 2, 64]),
               ALU.mult)
            g_v = G[:].rearrange("p k (j c) -> p k j c", c=16)
            TT(g_v,
               HS[:].unsqueeze(2).to_broadcast([128, KP, CNT_MAX, 16]),
               MS[:].unsqueeze(3).to_broadcast([128, KP, CNT_MAX, 16]),
               ALU.mult)

            for g5 in range(KP // 5):
                pa = ppa.tile([128, 5 * JC], f32, tag="pa")
                for q in range(5):
                    kp = 5 * g5 + q
                    nc.tensor.matmul(pa[:, q * JC:(q + 1) * JC],
                                     PT[:, kp, :], G[:, kp, :],
                                     start=True, stop=True)
                nc.scalar.activation(
                    AS[:, 5 * g5:5 * g5 + 5, :],
                    pa[:].rearrange("p (k f) -> p k f", f=JC),
                    AF.Square)

            H = KP // 2
            as_v = AS[:].rearrange("p k (j c) -> p k j c", c=16)
            for grp in range(2):
                pe = pph.tile([2, H * CNT_MAX], f32, tag="pe")
                for c in range(16):
                    nc.tensor.matmul(
                        pe[:],
                        SE[:, c, :],
                        as_v[:, grp * H:(grp + 1) * H, :, c],
                        start=(c == 0), stop=(c == 15))
                cs = grp * H * CNT_MAX
                nc.scalar.copy(OUTE[:, cs:cs + H * CNT_MAX], pe[:])
            nc.sync.dma_start(OUTE_d[:], OUTE[:])
    nc.compile()
    return nc


def kernel(positions, embed, W_rad, W_inv1, W_inv2, w_out, comp_weights,
           senders, receivers, species, structure_ids):
    from concourse import bass_utils

    positions = np.asarray(positions, np.float32)
    embed = np.asarray(embed, np.float32)
    W_rad = np.asarray(W_rad, np.float32)
    W_inv1 = np.asarray(W_inv1, np.float32)
    W_inv2 = np.asarray(W_inv2, np.float32)
    w_out = np.asarray(w_out, np.float32)
    comp_weights = np.asarray(comp_weights, np.float32)
    senders = np.asarray(senders).astype(np.int64)
    receivers = np.asarray(receivers).astype(np.int64)
    species = np.asarray(species).astype(np.int64)
    structure_ids_np = np.asarray(structure_ids).astype(np.int64)

    KP, tabs = _pack(senders, receivers)
    KPC = KP * CNT_MAX
    if KP not in _prog_cache:
        _prog_cache[KP] = (_build_A(KP), _build_B(KP))
    ncA, ncB = _prog_cache[KP]

    cemb = embed[species]  # [N,16]

    # stationary weight blocks
    def sw_pack(W):  # [256,16] -> [128, 16, 32] f16
        SW = np.zeros((128, 16, 32), np.float32)
        for b in range(2):
            rows = slice(b * 64, (b + 1) * 64)
            cols = slice(b * 16, (b + 1) * 16)
            for c in range(16):
                SW[rows, c, cols] = (_SFAC[:, None] *
                                     W[_LOF * 64 + _NOF * 16 + c, :])
        return SW.astype(np.float16)

    def se_pack(wo):  # [256] -> [128, 16, 2] f16
        SE = np.zeros((128, 16, 2), np.float32)
        for b in range(2):
            for c in range(16):
                SE[b * 64:(b + 1) * 64, c, b] = (
                    _SFAC * wo[_LOF * 64 + _NOF * 16 + c])
        return SE.astype(np.float16)

    SW1 = sw_pack(W_inv1)
    SE2 = se_pack(w_out)
    WRB = np.zeros((8, 16), np.float32)
    for l in range(L_MAX + 1):
        WRB[:, l * 4:(l + 1) * 4] = W_rad[l]
    WRB = np.broadcast_to(WRB[None], (128, 8, 16)).copy()

    maps1 = []
    for core in range(NCORES):
        tb = tabs[core]
        sl, val = tb["slot_send"], tb["slot_val"]
        amap = tb["amap"]
        bidx = tb["M2"].argmax(2)
        jidx = tb["MS"].argmax(2)
        ratom = amap[bidx, np.arange(KP)[None, :], jidx]
        ratom = np.where(val, ratom, 0)
        satom = np.where(val, sl, 0)
        pp = np.zeros((128, KP, 6), np.float32)
        pp[:, :, 0:3] = positions[satom]
        pp[:, :, 3:6] = positions[ratom]
        ce = np.zeros((32, KPC), np.float32)
        av = amap.reshape(2, KPC)
        for b in range(2):
            valid = av[b] >= 0
            ce[b * 16:(b + 1) * 16, valid] = cemb[av[b][valid]].T
        hs1 = cemb[satom]
        hs1[~val] = 0.0
        maps1.append(dict(pp=pp, wrb=WRB, hs=hs1.astype(np.float16),
                          msk=tb["MS"], m2=tb["M2"], sw=SW1, ce=ce))

    resA = bass_utils.run_bass_kernel_spmd(ncA, maps1,
                                           core_ids=list(range(NCORES)),
                                           trace=PROFILE)
    if PROFILE:
        LAST_PROF.append(resA)

    # assemble h1 and gather for layer 2
    h1_full = np.zeros((N_ATOMS, C), np.float32)
    for core in range(NCORES):
        amap = tabs[core]["amap"].reshape(2, KPC)
        outh = resA.results[core]["outh"]  # [32, KPC]
        for b in range(2):
            valid = amap[b] >= 0
            h1_full[amap[b][valid]] = outh[b * 16:(b + 1) * 16, valid].T

    maps2 = []
    for core in range(NCORES):
        tb = tabs[core]
        sl, val = tb["slot_send"], tb["slot_val"]
        hs2 = h1_full[np.where(val, sl, 0)]
        hs2[~val] = 0.0
        maps2.append(dict(pt0=resA.results[core]["pt0"],
                          hs=hs2.astype(np.float16),
                          msk=tb["MS"], m2=tb["M2"], se=SE2))
    resB = bass_utils.run_bass_kernel_spmd(ncB, maps2,
                                           core_ids=list(range(NCORES)),
                                           trace=PROFILE)
    if PROFILE:
        LAST_PROF.append(resB)

    e_atom = np.zeros(N_ATOMS, np.float32)
    for core in range(NCORES):
        amap = tabs[core]["amap"].reshape(2, KPC)
        oute = resB.results[core]["oute"]  # [2, KPC]
        for b in range(2):
            valid = amap[b] >= 0
            e_atom[amap[b][valid]] = oute[b, valid]
    e_atom += comp_weights[species]
    out = np.zeros(N_STRUCT, np.float32)
    np.add.at(out, structure_ids_np, e_atom)
    return out


# revision 6
# speedup vs baseline: 2.1813x; 1.0708x over previous
import sys
sys.path.insert(0, "/opt/trn_rl_repo")
import numpy as np

N_ATOMS = 10000
N_SPECIES = 8
N_STRUCT = 8
C = 16
N_BASIS = 8
L_MAX = 3
CUTOFF = 5.0
NCORES = 8
NC_AT = N_ATOMS // NCORES
CNT_MAX = 6
JC = CNT_MAX * C  # 96

_prog_cache = {}
PROFILE = False
LAST_PROF = []

# mn row order within a 64-row block: (l, m, n), n fastest
_LOF = np.repeat(np.arange(4), [(2 * l + 1) * 4 for l in range(4)])
_MOF = np.concatenate([np.repeat(np.arange(2 * l + 1), 4) for l in range(4)])
_NOF = np.concatenate([np.tile(np.arange(4), 2 * l + 1) for l in range(4)])
_SFAC = np.repeat([1.0 / np.sqrt(2.0 * l + 1.0) for l in range(4)],
                  [(2 * l + 1) * 4 for l in range(4)]).astype(np.float64)
# device emits raw sh polynomials; true sh = t[m] * raw (sign irrelevant,
# squares only). t^2 folded into SW/SE stationaries host-side.
_T = np.array([0.28209479,
               0.48860251, 0.48860251, 0.48860251,
               1.09254843, 1.09254843, 3 * 0.31539157, 1.09254843,
               0.54627422,
               3 * 0.59004359, 2.89061144, 5 * 0.45704579,
               5 * 0.37317633, 5 * 0.45704579, 1.44530572, 0.59004359],
              np.float64)
_GM = (_LOF * _LOF + _MOF)  # global m index per mn row
_TSQ = (_T[_GM] ** 2).astype(np.float64)


def _pack(senders, receivers):
    """FFD pack receiver atoms into pairs (2 blocks of <=CNT_MAX atoms,
    <=128 edges per pair)."""
    recv = np.asarray(receivers).astype(np.int64)
    send = np.asarray(senders).astype(np.int64)
    order = np.argsort(recv, kind="stable")
    ss = send[order]
    deg = np.bincount(recv, minlength=N_ATOMS)
    starts = np.zeros(N_ATOMS + 1, np.int64)
    starts[1:] = np.cumsum(deg)
    core_pairs = []
    for core in range(NCORES):
        a0 = core * NC_AT
        atoms = sorted(range(a0, a0 + NC_AT), key=lambda a: -deg[a])
        pairs = []
        for a in atoms:
            placed = False
            for p in pairs:
                if p[0] + deg[a] <= 128:
                    if len(p[1]) < CNT_MAX:
                        p[1].append(a); p[0] += deg[a]; placed = True; break
                    elif len(p[2]) < CNT_MAX:
                        p[2].append(a); p[0] += deg[a]; placed = True; break
            if not placed:
                pairs.append([deg[a], [a], []])
        core_pairs.append(pairs)
    KP = max(len(p) for p in core_pairs)
    if KP % 20:
        KP += 20 - KP % 20  # quarters divisible by 5
    tabs = []
    for core in range(NCORES):
        pairs = core_pairs[core]
        slot_send = np.zeros((128, KP), np.int64)
        slot_val = np.zeros((128, KP), bool)
        MS = np.zeros((128, KP, CNT_MAX), np.float16)
        M2 = np.zeros((128, KP, 2), np.float16)
        amap = np.full((2, KP, CNT_MAX), -1, np.int64)
        for kp, pr in enumerate(pairs):
            row = 0
            for b in range(2):
                for j, a in enumerate(pr[1 + b]):
                    s0, s1 = starts[a], starts[a + 1]
                    n = s1 - s0
                    slot_send[row:row + n, kp] = ss[s0:s1]
                    slot_val[row:row + n, kp] = True
                    MS[row:row + n, kp, j] = 1.0
                    M2[row:row + n, kp, b] = 1.0
                    amap[b, kp, j] = a
                    row += n
        tabs.append(dict(slot_send=slot_send, slot_val=slot_val,
                         MS=MS, M2=M2, amap=amap))
    return KP, tabs


def _emit_scatter(nc, tile, mybir, ppa, PT, G, AS2, KP, k0, k1):
    """Scatter matmuls + transposed squares for kp in [k0, k1)."""
    AF = mybir.ActivationFunctionType
    for g5 in range(k0 // 5, k1 // 5):
        pa = ppa.tile([128, 5 * JC], mybir.dt.float32, tag="pa")
        for q in range(5):
            kp = 5 * g5 + q
            nc.tensor.matmul(pa[:, q * JC:(q + 1) * JC],
                             PT[:, kp, :], G[:, kp, :],
                             start=True, stop=True)
        # transposed write: AS2[p, c, kp*6+j] = pa[p, (q, c, j)]^2
        dst = AS2[:, :, 5 * g5 * CNT_MAX:(5 * g5 + 5) * CNT_MAX].rearrange(
            "p c (k j) -> p k c j", k=5)
        nc.scalar.activation(
            dst, pa[:].rearrange("p (k c j) -> p k c j", k=5, c=16),
            AF.Square)


def _build_A(KP):
    import concourse.bass as bass
    import concourse.bacc as bacc
    import concourse.tile as tile
    from concourse import mybir

    f32 = mybir.dt.float32
    f16 = mybir.dt.float16
    ALU = mybir.AluOpType
    AF = mybir.ActivationFunctionType
    KPC = KP * CNT_MAX
    H2 = KP // 2

    nc = bacc.Bacc("TRN2", target_bir_lowering=False, debug=False,
                   num_devices=NCORES)
    PP_d = nc.dram_tensor("pp", [128, KP, 6], f32, kind="ExternalInput").ap()
    WR_d = nc.dram_tensor("wrb", [128, 8, 16], f16, kind="ExternalInput").ap()
    HS_d = nc.dram_tensor("hs", [128, KP, 16], f16, kind="ExternalInput").ap()
    MS_d = nc.dram_tensor("msk", [128, KP, CNT_MAX], f16,
                          kind="ExternalInput").ap()
    M2_d = nc.dram_tensor("m2", [128, KP, 2], f16, kind="ExternalInput").ap()
    SW_d = nc.dram_tensor("sw", [128, 16, 32], f16, kind="ExternalInput").ap()
    CE_d = nc.dram_tensor("ce", [32, KPC], f32, kind="ExternalInput").ap()
    PT0_d = nc.dram_tensor("pt0", [128, KP, 64], f16,
                           kind="ExternalOutput").ap()
    OUTH_d = nc.dram_tensor("outh", [32, KPC], f32,
                            kind="ExternalOutput").ap()

    with tile.TileContext(nc) as tc:
        with tc.tile_pool(name="main", bufs=1) as pool, \
             tc.tile_pool(name="pa", bufs=3, space="PSUM") as ppa, \
             tc.tile_pool(name="ph", bufs=2, space="PSUM") as pph:
            PP = pool.tile([128, KP, 6], f32, tag="pp")
            WR = pool.tile([128, 8, 16], f16, tag="wr")
            HS = pool.tile([128, KP, 16], f16, tag="hs")
            MS = pool.tile([128, KP, CNT_MAX], f16, tag="ms")
            M2 = pool.tile([128, KP, 2], f16, tag="m2")
            SW = pool.tile([128, 16, 32], f16, tag="sw")
            CE = pool.tile([32, KPC], f32, tag="ce")
            for t, dd in ((PP, PP_d), (WR, WR_d), (HS, HS_d), (MS, MS_d),
                          (M2, M2_d), (SW, SW_d), (CE, CE_d)):
                nc.sync.dma_start(t[:], dd[:])

            RV = pool.tile([128, KP, 3], f32, tag="rv")
            U = pool.tile([128, KP, 3], f32, tag="u")
            U16 = pool.tile([128, KP, 3], f16, tag="u16")
            SC = pool.tile([128, KP, 12], f32, tag="sc")
            SH = pool.tile([128, KP, 12], f16, tag="sh")  # raw sh m=4..15
            RR16 = pool.tile([128, KP, 16], f16, tag="rr")
            TM16 = pool.tile([128, KP, 16], f16, tag="tm")
            BB = pool.tile([128, KP, 8], f16, tag="bb")
            PT0 = pool.tile([128, KP, 64], f16, tag="pt0")
            PT = pool.tile([128, KP, 128], f16, tag="pt")
            G = pool.tile([128, KP, JC], f16, tag="g")
            AS2 = pool.tile([128, 16, KPC], f16, tag="as")
            OUTH = pool.tile([32, KPC], f32, tag="oh")

            def sc(i):
                return SC[:, :, i]

            TT = nc.vector.tensor_tensor
            TS = nc.vector.tensor_scalar
            GT = nc.gpsimd.tensor_tensor

            # geometry (positions pre-permuted host-side to (y,z,x))
            TT(RV[:], PP[:, :, 3:6], PP[:, :, 0:3], ALU.subtract)
            nc.vector.tensor_mul(U[:], RV[:], RV[:])
            nc.vector.reduce_sum(SC[:, :, 0:1], U[:], mybir.AxisListType.X)
            nc.scalar.activation(sc(1), sc(0), AF.Sqrt)          # r
            nc.vector.tensor_scalar_max(sc(2), sc(1), 1e-6)      # rc
            nc.vector.reciprocal(sc(3), sc(2))                   # rinv
            TT(U[:], RV[:], SC[:, :, 3:4].to_broadcast([128, KP, 3]),
               ALU.mult)
            nc.scalar.copy(U16[:], U[:])
            # fc = 0.5*cos(pi*min(r,5)/5)+0.5 ;  cos(x) = -sin(x - pi/2)
            nc.vector.tensor_scalar_min(sc(6), sc(1), CUTOFF)
            TS(sc(6), sc(6), float(np.pi / CUTOFF), float(-np.pi / 2),
               ALU.mult, ALU.add)
            nc.scalar.activation(sc(7), sc(6), AF.Sin)
            TS(sc(4), sc(7), -0.5, 0.5, ALU.mult, ALU.add)       # fc
            TT(sc(5), sc(4), sc(3), ALU.mult)
            nc.vector.tensor_scalar_mul(sc(5), sc(5),
                                        float(np.sqrt(2.0 / CUTOFF)))  # g
            y, z, x = U[:, :, 0], U[:, :, 1], U[:, :, 2]
            x2, y2, z2, xy, yz, xz = (sc(i) for i in (6, 7, 8, 9, 10, 11))
            nc.vector.tensor_mul(x2, x, x)
            nc.vector.tensor_mul(y2, y, y)
            nc.vector.tensor_mul(z2, z, z)
            nc.vector.tensor_mul(xy, x, y)
            nc.vector.tensor_mul(yz, y, z)
            nc.vector.tensor_mul(xz, x, z)

            # raw sh m=4..15 -> SH cols 0..11
            def shm(m):
                return SH[:, :, m - 4]

            nc.scalar.copy(shm(4), xy)
            nc.scalar.copy(shm(5), yz)
            nc.vector.tensor_scalar_add(shm(6), z2, -1.0 / 3.0)
            nc.scalar.copy(shm(7), xz)
            d_, t_ = sc(0), sc(1)
            TT(d_, x2, y2, ALU.subtract)                  # x2-y2
            nc.scalar.copy(shm(8), d_)
            nc.vector.scalar_tensor_tensor(t_, y2, 1.0 / 3.0, x2,
                                           ALU.mult, ALU.subtract)
            TT(shm(9), t_, y, ALU.mult)                   # y*(y2/3-x2)
            TT(shm(10), xy, z, ALU.mult)                  # xyz
            nc.vector.tensor_scalar_add(t_, z2, -0.2)
            TT(shm(11), t_, y, ALU.mult)                  # y*(z2-1/5)
            TT(shm(13), t_, x, ALU.mult)                  # x*(z2-1/5)
            nc.vector.tensor_scalar_add(t_, z2, -0.6)
            TT(shm(12), t_, z, ALU.mult)                  # z*(z2-3/5)
            TT(shm(14), d_, z, ALU.mult)                  # z*(x2-y2)
            nc.vector.scalar_tensor_tensor(t_, y2, 3.0, x2,
                                           ALU.mult, ALU.subtract)
            TT(shm(15), t_, x, ALU.mult)                  # x*(3y2-x2)

            # radial: s_b chain (f32 smalls) + f16 bb per b
            C2, SA, SB, TP = sc(0), sc(1), sc(3), sc(4)
            TS(sc(6), sc(2), float(np.pi / CUTOFF), float(-np.pi),
               ALU.mult, ALU.add)
            nc.scalar.activation(sc(7), sc(6), AF.Sin)
            nc.vector.tensor_scalar_mul(SA, sc(7), -1.0)          # s1
            TS(sc(6), sc(2), float(np.pi / CUTOFF), float(-np.pi / 2),
               ALU.mult, ALU.add)
            nc.scalar.activation(sc(7), sc(6), AF.Sin)
            nc.vector.tensor_scalar_mul(C2, sc(7), -2.0)          # 2cos
            for b in range(1, N_BASIS + 1):
                if b == 1:
                    cur = SA
                elif b == 2:
                    TT(SB, C2, SA, ALU.mult)
                    cur = SB
                else:
                    TT(TP, C2, SB if b % 2 else SA, ALU.mult)
                    dst = SA if b % 2 else SB
                    TT(dst, TP, SA if b % 2 else SB, ALU.subtract)
                    cur = dst
                TT(sc(7), cur, sc(5), ALU.mult)
                nc.scalar.copy(BB[:, :, b - 1], sc(7))

            def half_chain(h):
                sl = slice(h * H2, (h + 1) * H2)
                # radial accumulate (f16)
                for b in range(1, N_BASIS + 1):
                    bb = BB[:, sl, b - 1].unsqueeze(2).to_broadcast(
                        [128, H2, 16])
                    wb = WR[:, b - 1:b, :].to_broadcast([128, H2, 16])
                    if b == 1:
                        TT(RR16[:, sl, :], bb, wb, ALU.mult)
                    else:
                        TT(TM16[:, sl, :], bb, wb, ALU.mult)
                        TT(RR16[:, sl, :], RR16[:, sl, :], TM16[:, sl, :],
                           ALU.add)
                # PT0: l=0 copy, l=1 from U16, l=2/3 from SH
                nc.scalar.copy(PT0[:, sl, 0:4], RR16[:, sl, 0:4])
                TT(PT0[:, sl, 4:16].rearrange("p k (m n) -> p k m n", n=4),
                   U16[:, sl, :].unsqueeze(3).to_broadcast([128, H2, 3, 4]),
                   RR16[:, sl, 4:8].unsqueeze(2).to_broadcast([128, H2, 3, 4]),
                   ALU.mult)
                TT(PT0[:, sl, 16:36].rearrange("p k (m n) -> p k m n", n=4),
                   SH[:, sl, 0:5].unsqueeze(3).to_broadcast([128, H2, 5, 4]),
                   RR16[:, sl, 8:12].unsqueeze(2).to_broadcast(
                       [128, H2, 5, 4]),
                   ALU.mult)
                TT(PT0[:, sl, 36:64].rearrange("p k (m n) -> p k m n", n=4),
                   SH[:, sl, 5:12].unsqueeze(3).to_broadcast([128, H2, 7, 4]),
                   RR16[:, sl, 12:16].unsqueeze(2).to_broadcast(
                       [128, H2, 7, 4]),
                   ALU.mult)
                # block-diag expand
                TT(PT[:, sl, :].rearrange("p k (b m) -> p k b m", b=2),
                   PT0[:, sl, :].unsqueeze(2).to_broadcast([128, H2, 2, 64]),
                   M2[:, sl, :].unsqueeze(3).to_broadcast([128, H2, 2, 64]),
                   ALU.mult)

            # G on gpsimd (independent of geometry)
            GT(G[:].rearrange("p k (c j) -> p k c j", j=CNT_MAX),
               HS[:].unsqueeze(3).to_broadcast([128, KP, 16, CNT_MAX]),
               MS[:].unsqueeze(2).to_broadcast([128, KP, 16, CNT_MAX]),
               ALU.mult)

            for h in range(2):
                sl = slice(h * H2, (h + 1) * H2)
                half_chain(h)
                nc.sync.dma_start(PT0_d[:, sl, :], PT0[:, sl, :])
                _emit_scatter(nc, tile, mybir, ppa, PT, G, AS2, KP,
                              h * H2, (h + 1) * H2)
                # output stage for this half
                ph = pph.tile([32, H2 * CNT_MAX], f32, tag="ph")
                for c in range(16):
                    nc.tensor.matmul(
                        ph[:], SW[:, c, :],
                        AS2[:, c, h * H2 * CNT_MAX:(h + 1) * H2 * CNT_MAX],
                        start=(c == 0), stop=(c == 15))
                cs = h * H2 * CNT_MAX
                TT(OUTH[:, cs:cs + H2 * CNT_MAX], ph[:],
                   CE[:, cs:cs + H2 * CNT_MAX], ALU.mult)
                nc.sync.dma_start(OUTH_d[:, cs:cs + H2 * CNT_MAX],
                                  OUTH[:, cs:cs + H2 * CNT_MAX])
    nc.compile()
    return nc


def _build_B(KP):
    import concourse.bass as bass
    import concourse.bacc as bacc
    import concourse.tile as tile
    from concourse import mybir

    f32 = mybir.dt.float32
    f16 = mybir.dt.float16
    ALU = mybir.AluOpType
    KPC = KP * CNT_MAX
    Q4 = KP // 4

    nc = bacc.Bacc("TRN2", target_bir_lowering=False, debug=False,
                   num_devices=NCORES)
    PT0_d = nc.dram_tensor("pt0", [128, KP, 64], f16,
                           kind="ExternalInput").ap()
    HS_d = nc.dram_tensor("hs", [128, KP, 16], f16, kind="ExternalInput").ap()
    MS_d = nc.dram_tensor("msk", [128, KP, CNT_MAX], f16,
                          kind="ExternalInput").ap()
    M2_d = nc.dram_tensor("m2", [128, KP, 2], f16, kind="ExternalInput").ap()
    SE_d = nc.dram_tensor("se", [128, 16, 2], f16, kind="ExternalInput").ap()
    OUTE_d = nc.dram_tensor("oute", [2, KPC], f32,
                            kind="ExternalOutput").ap()

    with tile.TileContext(nc) as tc:
        with tc.tile_pool(name="main", bufs=1) as pool, \
             tc.tile_pool(name="pa", bufs=3, space="PSUM") as ppa, \
             tc.tile_pool(name="ph", bufs=2, space="PSUM") as pph:
            PT0 = pool.tile([128, KP, 64], f16, tag="pt0")
            HS = pool.tile([128, KP, 16], f16, tag="hs")
            MS = pool.tile([128, KP, CNT_MAX], f16, tag="ms")
            M2 = pool.tile([128, KP, 2], f16, tag="m2")
            SE = pool.tile([128, 16, 2], f16, tag="se")
            PT = pool.tile([128, KP, 128], f16, tag="pt")
            G = pool.tile([128, KP, JC], f16, tag="g")
            AS2 = pool.tile([128, 16, KPC], f16, tag="as")
            OUTE = pool.tile([2, KPC], f32, tag="oe")

            nc.sync.dma_start(SE[:], SE_d[:])
            TT = nc.vector.tensor_tensor
            GT = nc.gpsimd.tensor_tensor
            for q in range(4):
                sl = slice(q * Q4, (q + 1) * Q4)
                for t, dd in ((PT0, PT0_d), (HS, HS_d), (MS, MS_d),
                              (M2, M2_d)):
                    nc.sync.dma_start(t[:, sl], dd[:, sl])
                TT(PT[:, sl, :].rearrange("p k (b m) -> p k b m", b=2),
                   PT0[:, sl, :].unsqueeze(2).to_broadcast([128, Q4, 2, 64]),
                   M2[:, sl, :].unsqueeze(3).to_broadcast([128, Q4, 2, 64]),
                   ALU.mult)
                GT(G[:, sl, :].rearrange("p k (c j) -> p k c j", j=CNT_MAX),
                   HS[:, sl, :].unsqueeze(3).to_broadcast(
                       [128, Q4, 16, CNT_MAX]),
                   MS[:, sl, :].unsqueeze(2).to_broadcast(
                       [128, Q4, 16, CNT_MAX]),
                   ALU.mult)
                _emit_scatter(nc, tile, mybir, ppa, PT, G, AS2, KP,
                              q * Q4, (q + 1) * Q4)
                if q % 2 == 1:
                    h = q // 2
                    H2 = KP // 2
                    pe = pph.tile([2, H2 * CNT_MAX], f32, tag="pe")
                    for c in range(16):
                        nc.tensor.matmul(
                            pe[:], SE[:, c, :],
                            AS2[:, c,
                                h * H2 * CNT_MAX:(h + 1) * H2 * CNT_MAX],
                            start=(c == 0), stop=(c == 15))
                    cs = h * H2 * CNT_MAX
                    nc.scalar.copy(OUTE[:, cs:cs + H2 * CNT_MAX], pe[:])
                    nc.sync.dma_start(OUTE_d[:, cs:cs + H2 * CNT_MAX],
                                      OUTE[:, cs:cs + H2 * CNT_MAX])
    nc.compile()
    return nc


def kernel(positions, embed, W_rad, W_inv1, W_inv2, w_out, comp_weights,
           senders, receivers, species, structure_ids):
    from concourse import bass_utils

    positions = np.asarray(positions, np.float32)
    embed = np.asarray(embed, np.float32)
    W_rad = np.asarray(W_rad, np.float32)
    W_inv1 = np.asarray(W_inv1, np.float32)
    W_inv2 = np.asarray(W_inv2, np.float32)
    w_out = np.asarray(w_out, np.float32)
    comp_weights = np.asarray(comp_weights, np.float32)
    senders = np.asarray(senders).astype(np.int64)
    receivers = np.asarray(receivers).astype(np.int64)
    species = np.asarray(species).astype(np.int64)
    structure_ids_np = np.asarray(structure_ids).astype(np.int64)

    KP, tabs = _pack(senders, receivers)
    KPC = KP * CNT_MAX
    if KP not in _prog_cache:
        _prog_cache[KP] = (_build_A(KP), _build_B(KP))
    ncA, ncB = _prog_cache[KP]

    cemb = embed[species]  # [N,16]

    def sw_pack(W):  # [256,16] -> [128, 16, 32] f16
        SW = np.zeros((128, 16, 32), np.float64)
        for b in range(2):
            rows = slice(b * 64, (b + 1) * 64)
            cols = slice(b * 16, (b + 1) * 16)
            for c in range(16):
                SW[rows, c, cols] = ((_SFAC * _TSQ)[:, None] *
                                     W[_LOF * 64 + _NOF * 16 + c, :])
        return SW.astype(np.float16)

    def se_pack(wo):  # [256] -> [128, 16, 2] f16
        SE = np.zeros((128, 16, 2), np.float64)
        for b in range(2):
            for c in range(16):
                SE[b * 64:(b + 1) * 64, c, b] = (
                    _SFAC * _TSQ * wo[_LOF * 64 + _NOF * 16 + c])
        return SE.astype(np.float16)

    SW1 = sw_pack(W_inv1)
    SE2 = se_pack(w_out)
    WRB = np.zeros((8, 16), np.float32)
    for l in range(L_MAX + 1):
        WRB[:, l * 4:(l + 1) * 4] = W_rad[l]
    WRB = np.broadcast_to(WRB[None], (128, 8, 16)).astype(np.float16).copy()

    PERM = np.array([1, 2, 0])  # (x,y,z) -> (y,z,x)
    maps1 = []
    for core in range(NCORES):
        tb = tabs[core]
        sl, val = tb["slot_send"], tb["slot_val"]
        amap = tb["amap"]
        bidx = tb["M2"].argmax(2)
        jidx = tb["MS"].argmax(2)
        ratom = amap[bidx, np.arange(KP)[None, :], jidx]
        ratom = np.where(val, ratom, 0)
        satom = np.where(val, sl, 0)
        pp = np.zeros((128, KP, 6), np.float32)
        pp[:, :, 0:3] = positions[satom][:, :, PERM]
        pp[:, :, 3:6] = positions[ratom][:, :, PERM]
        ce = np.zeros((32, KPC), np.float32)
        av = amap.reshape(2, KPC)
        for b in range(2):
            valid = av[b] >= 0
            ce[b * 16:(b + 1) * 16, valid] = cemb[av[b][valid]].T
        hs1 = cemb[satom]
        hs1[~val] = 0.0
        maps1.append(dict(pp=pp, wrb=WRB, hs=hs1.astype(np.float16),
                          msk=tb["MS"], m2=tb["M2"], sw=SW1, ce=ce))

    resA = bass_utils.run_bass_kernel_spmd(ncA, maps1,
                                           core_ids=list(range(NCORES)),
                                           trace=PROFILE)
    if PROFILE:
        LAST_PROF.append(resA)

    h1_full = np.zeros((N_ATOMS, C), np.float32)
    for core in range(NCORES):
        amap = tabs[core]["amap"].reshape(2, KPC)
        outh = resA.results[core]["outh"]  # [32, KPC]
        for b in range(2):
            valid = amap[b] >= 0
            h1_full[amap[b][valid]] = outh[b * 16:(b + 1) * 16, valid].T

    maps2 = []
    for core in range(NCORES):
        tb = tabs[core]
        sl, val = tb["slot_send"], tb["slot_val"]
        hs2 = h1_full[np.where(val, sl, 0)]
        hs2[~val] = 0.0
        maps2.append(dict(pt0=resA.results[core]["pt0"],
                          hs=hs2.astype(np.float16),
                          msk=tb["MS"], m2=tb["M2"], se=SE2))
    resB = bass_utils.run_bass_kernel_spmd(ncB, maps2,
                                           core_ids=list(range(NCORES)),
                                           trace=PROFILE)
    if PROFILE:
        LAST_PROF.append(resB)

    e_atom = np.zeros(N_ATOMS, np.float32)
    for core in range(NCORES):
        amap = tabs[core]["amap"].reshape(2, KPC)
        oute = resB.results[core]["oute"]  # [2, KPC]
        for b in range(2):
            valid = amap[b] >= 0
            e_atom[amap[b][valid]] = oute[b, valid]
    e_atom += comp_weights[species]
    out = np.zeros(N_STRUCT, np.float32)
    np.add.at(out, structure_ids_np, e_atom)
    return out


# revision 15
# speedup vs baseline: 2.5072x; 1.1494x over previous
import sys
sys.path.insert(0, "/opt/trn_rl_repo")
import numpy as np

N_ATOMS = 10000
N_SPECIES = 8
N_STRUCT = 8
C = 16
N_BASIS = 8
L_MAX = 3
CUTOFF = 5.0
NCORES = 8
NC_AT = N_ATOMS // NCORES
CNT_MAX = 6
JC = CNT_MAX * C  # 96

_prog_cache = {}
PROFILE = False
LAST_PROF = []

# mn row order within a 64-row block: (l, m, n), n fastest
_LOF = np.repeat(np.arange(4), [(2 * l + 1) * 4 for l in range(4)])
_MOF = np.concatenate([np.repeat(np.arange(2 * l + 1), 4) for l in range(4)])
_NOF = np.concatenate([np.tile(np.arange(4), 2 * l + 1) for l in range(4)])
_SFAC = np.repeat([1.0 / np.sqrt(2.0 * l + 1.0) for l in range(4)],
                  [(2 * l + 1) * 4 for l in range(4)]).astype(np.float64)
# device emits raw sh polynomials; true sh = t[m] * raw (sign irrelevant,
# squares only). t^2 folded into SW/SE stationaries host-side.
_T = np.array([0.28209479,
               0.48860251, 0.48860251, 0.48860251,
               1.09254843, 1.09254843, 3 * 0.31539157, 1.09254843,
               0.54627422,
               3 * 0.59004359, 2.89061144, 5 * 0.45704579,
               5 * 0.37317633, 5 * 0.45704579, 1.44530572, 0.59004359],
              np.float64)
_GM = (_LOF * _LOF + _MOF)  # global m index per mn row
_TSQ = (_T[_GM] ** 2).astype(np.float64)


def _pack(senders, receivers):
    """FFD pack receiver atoms into pairs (2 blocks of <=CNT_MAX atoms,
    <=128 edges per pair)."""
    recv = np.asarray(receivers).astype(np.int64)
    send = np.asarray(senders).astype(np.int64)
    order = np.argsort(recv, kind="stable")
    ss = send[order]
    deg = np.bincount(recv, minlength=N_ATOMS)
    starts = np.zeros(N_ATOMS + 1, np.int64)
    starts[1:] = np.cumsum(deg)
    core_pairs = []
    for core in range(NCORES):
        a0 = core * NC_AT
        atoms = sorted(range(a0, a0 + NC_AT), key=lambda a: -deg[a])
        pairs = []
        for a in atoms:
            placed = False
            for p in pairs:
                if p[0] + deg[a] <= 128:
                    if len(p[1]) < CNT_MAX:
                        p[1].append(a); p[0] += deg[a]; placed = True; break
                    elif len(p[2]) < CNT_MAX:
                        p[2].append(a); p[0] += deg[a]; placed = True; break
            if not placed:
                pairs.append([deg[a], [a], []])
        core_pairs.append(pairs)
    KP = max(len(p) for p in core_pairs)
    if KP % 20:
        KP += 20 - KP % 20  # quarters divisible by 5
    tabs = []
    for core in range(NCORES):
        pairs = core_pairs[core]
        slot_send = np.zeros((128, KP), np.int64)
        slot_val = np.zeros((128, KP), bool)
        MS = np.zeros((128, KP, CNT_MAX), np.float32)
        M2 = np.zeros((128, KP, 2), np.float32)
        amap = np.full((2, KP, CNT_MAX), -1, np.int64)
        for kp, pr in enumerate(pairs):
            row = 0
            for b in range(2):
                for j, a in enumerate(pr[1 + b]):
                    s0, s1 = starts[a], starts[a + 1]
                    n = s1 - s0
                    slot_send[row:row + n, kp] = ss[s0:s1]
                    slot_val[row:row + n, kp] = True
                    MS[row:row + n, kp, j] = 1.0
                    M2[row:row + n, kp, b] = 1.0
                    amap[b, kp, j] = a
                    row += n
        tabs.append(dict(slot_send=slot_send, slot_val=slot_val,
                         MS=MS, M2=M2, amap=amap))
    return KP, tabs


def _emit_scatter(nc, tile, mybir, ppa, PT, G, AS2, KP, k0, k1):
    """Scatter matmuls + transposed squares for kp in [k0, k1)."""
    AF = mybir.ActivationFunctionType
    for g5 in range(k0 // 5, k1 // 5):
        pa = ppa.tile([128, 5 * JC], mybir.dt.float32, tag="pa")
        for q in range(5):
            kp = 5 * g5 + q
            nc.tensor.matmul(pa[:, q * JC:(q + 1) * JC],
                             PT[:, kp, :], G[:, kp, :],
                             start=True, stop=True)
        # transposed write: AS2[p, c, kp*6+j] = pa[p, (q, c, j)]^2
        dst = AS2[:, :, 5 * g5 * CNT_MAX:(5 * g5 + 5) * CNT_MAX].rearrange(
            "p c (k j) -> p k c j", k=5)
        nc.scalar.activation(
            dst, pa[:].rearrange("p (k c j) -> p k c j", k=5, c=16),
            AF.Square)


def _build_A(KP):
    import concourse.bass as bass
    import concourse.bacc as bacc
    import concourse.tile as tile
    from concourse import mybir

    f32 = mybir.dt.float32
    f16 = mybir.dt.float16
    ALU = mybir.AluOpType
    AF = mybir.ActivationFunctionType
    KPC = KP * CNT_MAX
    H2 = KP // 2

    nc = bacc.Bacc("TRN2", target_bir_lowering=False, debug=False,
                   num_devices=NCORES)
    PP_d = nc.dram_tensor("pp", [128, KP, 6], f32, kind="ExternalInput").ap()
    WR_d = nc.dram_tensor("wrb", [128, 8, 16], f32, kind="ExternalInput").ap()
    HS_d = nc.dram_tensor("hs", [128, KP, 16], f32, kind="ExternalInput").ap()
    MS_d = nc.dram_tensor("msk", [128, KP, CNT_MAX], f32,
                          kind="ExternalInput").ap()
    M2_d = nc.dram_tensor("m2", [128, KP, 2], f32, kind="ExternalInput").ap()
    SW_d = nc.dram_tensor("sw", [128, 16, 32], f16, kind="ExternalInput").ap()
    CE_d = nc.dram_tensor("ce", [32, KPC], f32, kind="ExternalInput").ap()
    PT0_d = nc.dram_tensor("pt0", [128, KP, 64], f32,
                           kind="ExternalOutput").ap()
    OUTH_d = nc.dram_tensor("outh", [32, KPC], f32,
                            kind="ExternalOutput").ap()

    with tile.TileContext(nc) as tc:
        with tc.tile_pool(name="main", bufs=1) as pool, \
             tc.tile_pool(name="pa", bufs=3, space="PSUM") as ppa, \
             tc.tile_pool(name="ph", bufs=2, space="PSUM") as pph:
            PP = pool.tile([128, KP, 6], f32, tag="pp")
            WR = pool.tile([128, 8, 16], f32, tag="wr")
            HS = pool.tile([128, KP, 16], f32, tag="hs")
            MS = pool.tile([128, KP, CNT_MAX], f32, tag="ms")
            M2 = pool.tile([128, KP, 2], f32, tag="m2")
            SW = pool.tile([128, 16, 32], f16, tag="sw")
            CE = pool.tile([32, KPC], f32, tag="ce")
            for t, dd in ((PP, PP_d), (WR, WR_d), (HS, HS_d), (MS, MS_d),
                          (M2, M2_d), (SW, SW_d), (CE, CE_d)):
                nc.sync.dma_start(t[:], dd[:])

            RV = pool.tile([128, KP, 3], f32, tag="rv")
            U = pool.tile([128, KP, 3], f32, tag="u")
            SC = pool.tile([128, KP, 12], f32, tag="sc")
            SH = pool.tile([128, KP, 12], f32, tag="sh")  # raw sh m=4..15
            RR = pool.tile([128, KP, 16], f32, tag="rr")
            TM = pool.tile([128, KP, 16], f32, tag="pp")  # reuse PP
            BB = pool.tile([128, KP, 8], f32, tag="rv")  # reuse RV
            PT0 = pool.tile([128, KP, 64], f32, tag="pt0")
            PT = pool.tile([128, KP, 128], f16, tag="pt")
            G = pool.tile([128, KP, JC], f16, tag="g")
            AS2 = pool.tile([128, 16, KPC], f16, tag="as")
            OUTH = pool.tile([32, KPC], f32, tag="sc")  # reuse SC

            def sc(i):
                return SC[:, :, i]

            TT = nc.vector.tensor_tensor
            TS = nc.vector.tensor_scalar
            GT = nc.gpsimd.tensor_tensor

            # geometry (positions pre-permuted host-side to (y,z,x))
            TT(RV[:], PP[:, :, 3:6], PP[:, :, 0:3], ALU.subtract)
            nc.vector.tensor_mul(U[:], RV[:], RV[:])
            nc.vector.reduce_sum(SC[:, :, 0:1], U[:], mybir.AxisListType.X)
            nc.scalar.activation(sc(1), sc(0), AF.Sqrt)          # r
            nc.vector.tensor_scalar_max(sc(2), sc(1), 1e-6)      # rc
            nc.vector.reciprocal(sc(3), sc(2))                   # rinv
            TT(U[:], RV[:], SC[:, :, 3:4].to_broadcast([128, KP, 3]),
               ALU.mult)
            # fc = 0.5*cos(pi*min(r,5)/5)+0.5 ;  cos(x) = -sin(x - pi/2)
            nc.vector.tensor_scalar_min(sc(6), sc(1), CUTOFF)
            TS(sc(6), sc(6), float(np.pi / CUTOFF), float(-np.pi / 2),
               ALU.mult, ALU.add)
            nc.scalar.activation(sc(7), sc(6), AF.Sin)
            TS(sc(4), sc(7), -0.5, 0.5, ALU.mult, ALU.add)       # fc
            TT(sc(5), sc(4), sc(3), ALU.mult)
            nc.vector.tensor_scalar_mul(sc(5), sc(5),
                                        float(np.sqrt(2.0 / CUTOFF)))  # g
            y, z, x = U[:, :, 0], U[:, :, 1], U[:, :, 2]
            x2, y2, z2, xy, yz, xz = (sc(i) for i in (6, 7, 8, 9, 10, 11))
            nc.vector.tensor_mul(x2, x, x)
            nc.vector.tensor_mul(y2, y, y)
            nc.vector.tensor_mul(z2, z, z)
            nc.vector.tensor_mul(xy, x, y)
            nc.vector.tensor_mul(yz, y, z)
            nc.vector.tensor_mul(xz, x, z)

            # raw sh m=4..15 -> SH cols 0..11
            def shm(m):
                return SH[:, :, m - 4]

            nc.scalar.copy(shm(4), xy)
            nc.scalar.copy(shm(5), yz)
            nc.vector.tensor_scalar_add(shm(6), z2, -1.0 / 3.0)
            nc.scalar.copy(shm(7), xz)
            d_, t_ = sc(0), sc(1)
            TT(d_, x2, y2, ALU.subtract)                  # x2-y2
            nc.scalar.copy(shm(8), d_)
            nc.vector.scalar_tensor_tensor(t_, y2, 1.0 / 3.0, x2,
                                           ALU.mult, ALU.subtract)
            TT(shm(9), t_, y, ALU.mult)                   # y*(y2/3-x2)
            TT(shm(10), xy, z, ALU.mult)                  # xyz
            nc.vector.tensor_scalar_add(t_, z2, -0.2)
            TT(shm(11), t_, y, ALU.mult)                  # y*(z2-1/5)
            TT(shm(13), t_, x, ALU.mult)                  # x*(z2-1/5)
            nc.vector.tensor_scalar_add(t_, z2, -0.6)
            TT(shm(12), t_, z, ALU.mult)                  # z*(z2-3/5)
            TT(shm(14), d_, z, ALU.mult)                  # z*(x2-y2)
            nc.vector.scalar_tensor_tensor(t_, y2, 3.0, x2,
                                           ALU.mult, ALU.subtract)
            TT(shm(15), t_, x, ALU.mult)                  # x*(3y2-x2)

            # radial: s_b chain (f32 smalls) + f16 bb per b
            C2, SA, SB, TP = sc(0), sc(1), sc(3), sc(4)
            TS(sc(6), sc(2), float(np.pi / CUTOFF), float(-np.pi),
               ALU.mult, ALU.add)
            nc.scalar.activation(sc(7), sc(6), AF.Sin)
            nc.vector.tensor_scalar_mul(SA, sc(7), -1.0)          # s1
            TS(sc(6), sc(2), float(np.pi / CUTOFF), float(-np.pi / 2),
               ALU.mult, ALU.add)
            nc.scalar.activation(sc(7), sc(6), AF.Sin)
            nc.vector.tensor_scalar_mul(C2, sc(7), -2.0)          # 2cos
            for b in range(1, N_BASIS + 1):
                if b == 1:
                    cur = SA
                elif b == 2:
                    TT(SB, C2, SA, ALU.mult)
                    cur = SB
                else:
                    TT(TP, C2, SB if b % 2 else SA, ALU.mult)
                    dst = SA if b % 2 else SB
                    TT(dst, TP, SA if b % 2 else SB, ALU.subtract)
                    cur = dst
                TT(sc(7), cur, sc(5), ALU.mult)
                nc.scalar.copy(BB[:, :, b - 1], sc(7))

            Q4 = KP // 4

            def quarter_chain(q):
                sl = slice(q * Q4, (q + 1) * Q4)
                # radial accumulate (f32)
                for b in range(1, N_BASIS + 1):
                    bb = BB[:, sl, b - 1].unsqueeze(2).to_broadcast(
                        [128, Q4, 16])
                    wb = WR[:, b - 1:b, :].to_broadcast([128, Q4, 16])
                    if b == 1:
                        TT(RR[:, sl, :], bb, wb, ALU.mult)
                    else:
                        TT(TM[:, sl, :], bb, wb, ALU.mult)
                        TT(RR[:, sl, :], RR[:, sl, :], TM[:, sl, :],
                           ALU.add)
                # PT0: l=0 copy, l=1 from U, l=2/3 from SH
                nc.scalar.copy(PT0[:, sl, 0:4], RR[:, sl, 0:4])
                TT(PT0[:, sl, 4:16].rearrange("p k (m n) -> p k m n", n=4),
                   U[:, sl, :].unsqueeze(3).to_broadcast([128, Q4, 3, 4]),
                   RR[:, sl, 4:8].unsqueeze(2).to_broadcast([128, Q4, 3, 4]),
                   ALU.mult)
                TT(PT0[:, sl, 16:36].rearrange("p k (m n) -> p k m n", n=4),
                   SH[:, sl, 0:5].unsqueeze(3).to_broadcast([128, Q4, 5, 4]),
                   RR[:, sl, 8:12].unsqueeze(2).to_broadcast(
                       [128, Q4, 5, 4]),
                   ALU.mult)
                TT(PT0[:, sl, 36:64].rearrange("p k (m n) -> p k m n", n=4),
                   SH[:, sl, 5:12].unsqueeze(3).to_broadcast([128, Q4, 7, 4]),
                   RR[:, sl, 12:16].unsqueeze(2).to_broadcast(
                       [128, Q4, 7, 4]),
                   ALU.mult)
                # block-diag expand
                TT(PT[:, sl, :].rearrange("p k (b m) -> p k b m", b=2),
                   PT0[:, sl, :].unsqueeze(2).to_broadcast([128, Q4, 2, 64]),
                   M2[:, sl, :].unsqueeze(3).to_broadcast([128, Q4, 2, 64]),
                   ALU.mult)

            # G on gpsimd (independent of geometry)
            GT(G[:].rearrange("p k (c j) -> p k c j", j=CNT_MAX),
               HS[:].unsqueeze(3).to_broadcast([128, KP, 16, CNT_MAX]),
               MS[:].unsqueeze(2).to_broadcast([128, KP, 16, CNT_MAX]),
               ALU.mult)

            phs = []
            for q in range(4):
                sl = slice(q * Q4, (q + 1) * Q4)
                quarter_chain(q)
                nc.sync.dma_start(PT0_d[:, sl, :], PT0[:, sl, :])
                _emit_scatter(nc, tile, mybir, ppa, PT, G, AS2, KP,
                              q * Q4, (q + 1) * Q4)
                if q % 2 == 1:
                    h = q // 2
                    ph = pph.tile([32, H2 * CNT_MAX], f32, tag="ph")
                    for c in range(16):
                        nc.tensor.matmul(
                            ph[:], SW[:, c, :],
                            AS2[:, c,
                                h * H2 * CNT_MAX:(h + 1) * H2 * CNT_MAX],
                            start=(c == 0), stop=(c == 15))
                    phs.append(ph)
            # h1 = h1_pre * cemb (vector; emitted after all chains so the
            # psum wait does not stall the quarter pipeline)
            for h, ph in enumerate(phs):
                cs = h * H2 * CNT_MAX
                TT(OUTH[:, cs:cs + H2 * CNT_MAX], ph[:],
                   CE[:, cs:cs + H2 * CNT_MAX], ALU.mult)
                nc.sync.dma_start(OUTH_d[:, cs:cs + H2 * CNT_MAX],
                                  OUTH[:, cs:cs + H2 * CNT_MAX])
    nc.compile()
    return nc


def _build_B(KP):
    import concourse.bass as bass
    import concourse.bacc as bacc
    import concourse.tile as tile
    from concourse import mybir

    f32 = mybir.dt.float32
    f16 = mybir.dt.float16
    ALU = mybir.AluOpType
    KPC = KP * CNT_MAX
    Q4 = KP // 4

    nc = bacc.Bacc("TRN2", target_bir_lowering=False, debug=False,
                   num_devices=NCORES)
    PT0_d = nc.dram_tensor("pt0", [128, KP, 64], f32,
                           kind="ExternalInput").ap()
    HS_d = nc.dram_tensor("hs", [128, KP, 16], f32, kind="ExternalInput").ap()
    MS_d = nc.dram_tensor("msk", [128, KP, CNT_MAX], f32,
                          kind="ExternalInput").ap()
    M2_d = nc.dram_tensor("m2", [128, KP, 2], f32, kind="ExternalInput").ap()
    SE_d = nc.dram_tensor("se", [128, 16, 2], f16, kind="ExternalInput").ap()
    OUTE_d = nc.dram_tensor("oute", [2, KPC], f32,
                            kind="ExternalOutput").ap()

    with tile.TileContext(nc) as tc:
        with tc.tile_pool(name="main", bufs=1) as pool, \
             tc.tile_pool(name="pa", bufs=3, space="PSUM") as ppa, \
             tc.tile_pool(name="ph", bufs=2, space="PSUM") as pph:
            PT0 = pool.tile([128, KP, 64], f32, tag="pt0")
            HS = pool.tile([128, KP, 16], f32, tag="hs")
            MS = pool.tile([128, KP, CNT_MAX], f32, tag="ms")
            M2 = pool.tile([128, KP, 2], f32, tag="m2")
            SE = pool.tile([128, 16, 2], f16, tag="se")
            PT = pool.tile([128, KP, 128], f16, tag="pt")
            G = pool.tile([128, KP, JC], f16, tag="g")
            AS2 = pool.tile([128, 16, KPC], f16, tag="as")
            OUTE = pool.tile([2, KPC], f32, tag="oe")

            nc.sync.dma_start(SE[:], SE_d[:])
            TT = nc.vector.tensor_tensor
            GT = nc.gpsimd.tensor_tensor
            for q in range(4):
                sl = slice(q * Q4, (q + 1) * Q4)
                for t, dd in ((PT0, PT0_d), (HS, HS_d), (MS, MS_d),
                              (M2, M2_d)):
                    nc.sync.dma_start(t[:, sl], dd[:, sl])
                TT(PT[:, sl, :].rearrange("p k (b m) -> p k b m", b=2),
                   PT0[:, sl, :].unsqueeze(2).to_broadcast([128, Q4, 2, 64]),
                   M2[:, sl, :].unsqueeze(3).to_broadcast([128, Q4, 2, 64]),
                   ALU.mult)
                GT(G[:, sl, :].rearrange("p k (c j) -> p k c j", j=CNT_MAX),
                   HS[:, sl, :].unsqueeze(3).to_broadcast(
                       [128, Q4, 16, CNT_MAX]),
                   MS[:, sl, :].unsqueeze(2).to_broadcast(
                       [128, Q4, 16, CNT_MAX]),
                   ALU.mult)
                _emit_scatter(nc, tile, mybir, ppa, PT, G, AS2, KP,
                              q * Q4, (q + 1) * Q4)
                if q % 2 == 1:
                    h = q // 2
                    H2 = KP // 2
                    pe = pph.tile([2, H2 * CNT_MAX], f32, tag="pe")
                    for c in range(16):
                        nc.tensor.matmul(
                            pe[:], SE[:, c, :],
                            AS2[:, c,
                                h * H2 * CNT_MAX:(h + 1) * H2 * CNT_MAX],
                            start=(c == 0), stop=(c == 15))
                    cs = h * H2 * CNT_MAX
                    nc.scalar.copy(OUTE[:, cs:cs + H2 * CNT_MAX], pe[:])
                    nc.sync.dma_start(OUTE_d[:, cs:cs + H2 * CNT_MAX],
                                      OUTE[:, cs:cs + H2 * CNT_MAX])
    nc.compile()
    return nc


def kernel(positions, embed, W_rad, W_inv1, W_inv2, w_out, comp_weights,
           senders, receivers, species, structure_ids):
    from concourse import bass_utils

    positions = np.asarray(positions, np.float32)
    embed = np.asarray(embed, np.float32)
    W_rad = np.asarray(W_rad, np.float32)
    W_inv1 = np.asarray(W_inv1, np.float32)
    W_inv2 = np.asarray(W_inv2, np.float32)
    w_out = np.asarray(w_out, np.float32)
    comp_weights = np.asarray(comp_weights, np.float32)
    senders = np.asarray(senders).astype(np.int64)
    receivers = np.asarray(receivers).astype(np.int64)
    species = np.asarray(species).astype(np.int64)
    structure_ids_np = np.asarray(structure_ids).astype(np.int64)

    KP, tabs = _pack(senders, receivers)
    KPC = KP * CNT_MAX
    if KP not in _prog_cache:
        _prog_cache[KP] = (_build_A(KP), _build_B(KP))
    ncA, ncB = _prog_cache[KP]

    cemb = embed[species]  # [N,16]

    def sw_pack(W):  # [256,16] -> [128, 16, 32] f16
        SW = np.zeros((128, 16, 32), np.float64)
        for b in range(2):
            rows = slice(b * 64, (b + 1) * 64)
            cols = slice(b * 16, (b + 1) * 16)
            for c in range(16):
                SW[rows, c, cols] = ((_SFAC * _TSQ)[:, None] *
                                     W[_LOF * 64 + _NOF * 16 + c, :])
        return SW.astype(np.float16)

    def se_pack(wo):  # [256] -> [128, 16, 2] f16
        SE = np.zeros((128, 16, 2), np.float64)
        for b in range(2):
            for c in range(16):
                SE[b * 64:(b + 1) * 64, c, b] = (
                    _SFAC * _TSQ * wo[_LOF * 64 + _NOF * 16 + c])
        return SE.astype(np.float16)

    SW1 = sw_pack(W_inv1)
    SE2 = se_pack(w_out)
    WRB = np.zeros((8, 16), np.float32)
    for l in range(L_MAX + 1):
        WRB[:, l * 4:(l + 1) * 4] = W_rad[l]
    WRB = np.broadcast_to(WRB[None], (128, 8, 16)).copy()

    PERM = np.array([1, 2, 0])  # (x,y,z) -> (y,z,x)
    maps1 = []
    for core in range(NCORES):
        tb = tabs[core]
        sl, val = tb["slot_send"], tb["slot_val"]
        amap = tb["amap"]
        bidx = tb["M2"].argmax(2)
        jidx = tb["MS"].argmax(2)
        ratom = amap[bidx, np.arange(KP)[None, :], jidx]
        ratom = np.where(val, ratom, 0)
        satom = np.where(val, sl, 0)
        pp = np.zeros((128, KP, 6), np.float32)
        pp[:, :, 0:3] = positions[satom][:, :, PERM]
        pp[:, :, 3:6] = positions[ratom][:, :, PERM]
        ce = np.zeros((32, KPC), np.float32)
        av = amap.reshape(2, KPC)
        for b in range(2):
            valid = av[b] >= 0
            ce[b * 16:(b + 1) * 16, valid] = cemb[av[b][valid]].T
        hs1 = cemb[satom]
        hs1[~val] = 0.0
        maps1.append(dict(pp=pp, wrb=WRB, hs=hs1,
                          msk=tb["MS"], m2=tb["M2"], sw=SW1, ce=ce))

    resA = bass_utils.run_bass_kernel_spmd(ncA, maps1,
                                           core_ids=list(range(NCORES)),
                                           trace=PROFILE)
    if PROFILE:
        LAST_PROF.append(resA)

    h1_full = np.zeros((N_ATOMS, C), np.float32)
    for core in range(NCORES):
        amap = tabs[core]["amap"].reshape(2, KPC)
        outh = resA.results[core]["outh"]  # [32, KPC]
        for b in range(2):
            valid = amap[b] >= 0
            h1_full[amap[b][valid]] = outh[b * 16:(b + 1) * 16, valid].T

    maps2 = []
    for core in range(NCORES):
        tb = tabs[core]
        sl, val = tb["slot_send"], tb["slot_val"]
        hs2 = h1_full[np.where(val, sl, 0)]
        hs2[~val] = 0.0
        maps2.append(dict(pt0=resA.results[core]["pt0"],
                          hs=hs2,
                          msk=tb["MS"], m2=tb["M2"], se=SE2))
    resB = bass_utils.run_bass_kernel_spmd(ncB, maps2,
                                           core_ids=list(range(NCORES)),
                                           trace=PROFILE)
    if PROFILE:
        LAST_PROF.append(resB)

    e_atom = np.zeros(N_ATOMS, np.float32)
    for core in range(NCORES):
        amap = tabs[core]["amap"].reshape(2, KPC)
        oute = resB.results[core]["oute"]  # [2, KPC]
        for b in range(2):
            valid = amap[b] >= 0
            e_atom[amap[b][valid]] = oute[b, valid]
    e_atom += comp_weights[species]
    out = np.zeros(N_STRUCT, np.float32)
    np.add.at(out, structure_ids_np, e_atom)
    return out


# revision 16
# speedup vs baseline: 2.8672x; 1.1436x over previous
import sys
sys.path.insert(0, "/opt/trn_rl_repo")
import numpy as np

N_ATOMS = 10000
N_SPECIES = 8
N_STRUCT = 8
C = 16
N_BASIS = 8
L_MAX = 3
CUTOFF = 5.0
NCORES = 8
NC_AT = N_ATOMS // NCORES
CNT_MAX = 6
JC = CNT_MAX * C  # 96

_prog_cache = {}
PROFILE = False
LAST_PROF = []

# mn row order within a 64-row block: (l, m, n), n fastest
_LOF = np.repeat(np.arange(4), [(2 * l + 1) * 4 for l in range(4)])
_MOF = np.concatenate([np.repeat(np.arange(2 * l + 1), 4) for l in range(4)])
_NOF = np.concatenate([np.tile(np.arange(4), 2 * l + 1) for l in range(4)])
_SFAC = np.repeat([1.0 / np.sqrt(2.0 * l + 1.0) for l in range(4)],
                  [(2 * l + 1) * 4 for l in range(4)]).astype(np.float64)
# device emits raw sh polynomials; true sh = t[m] * raw (sign irrelevant,
# squares only). t^2 folded into SW/SE stationaries host-side.
_T = np.array([0.28209479,
               0.48860251, 0.48860251, 0.48860251,
               1.09254843, 1.09254843, 3 * 0.31539157, 1.09254843,
               0.54627422,
               3 * 0.59004359, 2.89061144, 5 * 0.45704579,
               5 * 0.37317633, 5 * 0.45704579, 1.44530572, 0.59004359],
              np.float64)
_GM = (_LOF * _LOF + _MOF)  # global m index per mn row
_TSQ = (_T[_GM] ** 2).astype(np.float64)


def _pack(senders, receivers):
    """FFD pack receiver atoms into blocks (<=64 edges, <=CNT_MAX atoms);
    pair blocks; block b of a pair owns edge rows [64b, 64b+edges)."""
    recv = np.asarray(receivers).astype(np.int64)
    send = np.asarray(senders).astype(np.int64)
    order = np.argsort(recv, kind="stable")
    ss = send[order]
    deg = np.bincount(recv, minlength=N_ATOMS)
    starts = np.zeros(N_ATOMS + 1, np.int64)
    starts[1:] = np.cumsum(deg)
    core_blocks = []
    for core in range(NCORES):
        a0 = core * NC_AT
        atoms = sorted(range(a0, a0 + NC_AT), key=lambda a: -deg[a])
        blocks = []
        for a in atoms:
            for blk in blocks:
                if blk[0] + deg[a] <= 64 and len(blk[1]) < CNT_MAX:
                    blk[1].append(a); blk[0] += deg[a]; break
            else:
                blocks.append([deg[a], [a]])
        core_blocks.append(blocks)
    KP = max((len(b) + 1) // 2 for b in core_blocks)
    if KP % 4:
        KP += 4 - KP % 4
    tabs = []
    for core in range(NCORES):
        blocks = core_blocks[core]
        slot_send = np.zeros((128, KP), np.int64)
        slot_val = np.zeros((128, KP), bool)
        MS = np.zeros((128, KP, CNT_MAX), np.float32)
        amap = np.full((2, KP, CNT_MAX), -1, np.int64)
        for bi, (ecnt, atoms) in enumerate(blocks):
            kp, b = bi // 2, bi % 2
            row = 64 * b
            for j, a in enumerate(atoms):
                s0, s1 = starts[a], starts[a + 1]
                n = s1 - s0
                slot_send[row:row + n, kp] = ss[s0:s1]
                slot_val[row:row + n, kp] = True
                MS[row:row + n, kp, j] = 1.0
                amap[b, kp, j] = a
                row += n
            assert row <= 64 * b + 64
        tabs.append(dict(slot_send=slot_send, slot_val=slot_val,
                         MS=MS, amap=amap))
    return KP, tabs


def _emit_scatter(nc, mybir, ppa, PT, G, AS2, k0, k1):
    """Scatter matmuls + transposed squares for kp in [k0, k1),
    ragged groups of 4."""
    AF = mybir.ActivationFunctionType
    f32 = mybir.dt.float32
    g0 = k0
    while g0 < k1:
        n = min(4, k1 - g0)
        pa = ppa.tile([128, 4 * JC], f32, tag="pa")
        for q in range(n):
            kp = g0 + q
            nc.tensor.matmul(pa[:, q * JC:(q + 1) * JC],
                             PT[:, kp, :], G[:, kp, :],
                             start=True, stop=True)
        dst = AS2[:, :, g0 * CNT_MAX:(g0 + n) * CNT_MAX].rearrange(
            "p c (k j) -> p k c j", k=n)
        nc.scalar.activation(
            dst, pa[:, 0:n * JC].rearrange("p (k c j) -> p k c j",
                                           k=n, c=16),
            AF.Square)
        g0 += n


def _build_A(KP):
    import concourse.bass as bass
    import concourse.bacc as bacc
    import concourse.tile as tile
    from concourse import mybir

    f32 = mybir.dt.float32
    f16 = mybir.dt.float16
    ALU = mybir.AluOpType
    AF = mybir.ActivationFunctionType
    KPC = KP * CNT_MAX
    H2 = KP // 2
    Q4 = KP // 4

    nc = bacc.Bacc("TRN2", target_bir_lowering=False, debug=False,
                   num_devices=NCORES)
    PP_d = nc.dram_tensor("pp", [128, KP, 6], f32, kind="ExternalInput").ap()
    WR_d = nc.dram_tensor("wrb", [128, 8, 16], f32,
                          kind="ExternalInput").ap()
    GX_d = nc.dram_tensor("gx", [128, KP, JC], f16,
                          kind="ExternalInput").ap()
    SW_d = nc.dram_tensor("sw", [128, 16, 32], f16, kind="ExternalInput").ap()
    CE_d = nc.dram_tensor("ce", [32, KPC], f32, kind="ExternalInput").ap()
    PT0_d = nc.dram_tensor("pt0", [128, KP, 64], f16,
                           kind="ExternalOutput").ap()
    OUTH_d = nc.dram_tensor("outh", [32, KPC], f32,
                            kind="ExternalOutput").ap()

    with tile.TileContext(nc) as tc:
        with tc.tile_pool(name="main", bufs=1) as pool, \
             tc.tile_pool(name="pa", bufs=3, space="PSUM") as ppa, \
             tc.tile_pool(name="ph", bufs=2, space="PSUM") as pph:
            PP = pool.tile([128, KP, 6], f32, tag="pp")
            WR = pool.tile([128, 8, 16], f32, tag="wr")
            GX = pool.tile([128, KP, JC], f16, tag="g")
            SW = pool.tile([128, 16, 32], f16, tag="sw")
            CE = pool.tile([32, KPC], f32, tag="ce")
            nc.sync.dma_start(PP[:], PP_d[:])
            nc.sync.dma_start(WR[:], WR_d[:])
            for q in range(4):
                sl = slice(q * Q4, (q + 1) * Q4)
                nc.sync.dma_start(GX[:, sl], GX_d[:, sl])
            nc.sync.dma_start(SW[:], SW_d[:])
            nc.sync.dma_start(CE[:], CE_d[:])

            RV = pool.tile([128, KP, 3], f32, tag="rv")
            U = pool.tile([128, KP, 3], f32, tag="u")
            SC = pool.tile([128, KP, 12], f32, tag="sc")
            SH = pool.tile([128, KP, 12], f32, tag="sh")  # raw sh m=4..15
            RR = pool.tile([128, KP, 16], f32, tag="rr")
            TM = pool.tile([128, KP, 16], f32, tag="pp2")
            BB = pool.tile([128, KP, 8], f32, tag="bb")
            PT0 = pool.tile([128, KP, 64], f16, tag="pt0")
            PT = pool.tile([128, KP, 128], f16, tag="pt")
            AS2 = pool.tile([128, 16, KPC], f16, tag="as")
            OUTH = pool.tile([32, KPC], f32, tag="oh")

            def sc(i):
                return SC[:, :, i]

            TT = nc.vector.tensor_tensor
            TS = nc.vector.tensor_scalar
            GT = nc.gpsimd.tensor_tensor

            # zero PT off-diagonal blocks once (gpsimd, off critical path)
            for q in range(4):
                sl = slice(q * Q4, (q + 1) * Q4)
                nc.gpsimd.memset(PT[:, sl, :], 0.0)

            # geometry (positions pre-permuted host-side to (y,z,x))
            TT(RV[:], PP[:, :, 3:6], PP[:, :, 0:3], ALU.subtract)
            nc.vector.tensor_mul(U[:], RV[:], RV[:])
            nc.vector.reduce_sum(SC[:, :, 0:1], U[:], mybir.AxisListType.X)
            nc.scalar.activation(sc(1), sc(0), AF.Sqrt)          # r
            nc.vector.tensor_scalar_max(sc(2), sc(1), 1e-6)      # rc
            nc.vector.reciprocal(sc(3), sc(2))                   # rinv
            TT(U[:], RV[:], SC[:, :, 3:4].to_broadcast([128, KP, 3]),
               ALU.mult)
            # fc = 0.5*cos(pi*min(r,5)/5)+0.5 ;  cos(x) = -sin(x - pi/2)
            nc.vector.tensor_scalar_min(sc(6), sc(1), CUTOFF)
            TS(sc(6), sc(6), float(np.pi / CUTOFF), float(-np.pi / 2),
               ALU.mult, ALU.add)
            nc.scalar.activation(sc(7), sc(6), AF.Sin)
            TS(sc(4), sc(7), -0.5, 0.5, ALU.mult, ALU.add)       # fc
            TT(sc(5), sc(4), sc(3), ALU.mult)
            nc.vector.tensor_scalar_mul(sc(5), sc(5),
                                        float(np.sqrt(2.0 / CUTOFF)))  # g
            y, z, x = U[:, :, 0], U[:, :, 1], U[:, :, 2]
            x2, y2, z2, xy, yz, xz = (sc(i) for i in (6, 7, 8, 9, 10, 11))
            nc.vector.tensor_mul(x2, x, x)
            nc.vector.tensor_mul(y2, y, y)
            nc.vector.tensor_mul(z2, z, z)
            nc.vector.tensor_mul(xy, x, y)
            nc.vector.tensor_mul(yz, y, z)
            nc.vector.tensor_mul(xz, x, z)

            # raw sh m=4..15 -> SH cols 0..11
            def shm(m):
                return SH[:, :, m - 4]

            nc.scalar.copy(shm(4), xy)
            nc.scalar.copy(shm(5), yz)
            nc.vector.tensor_scalar_add(shm(6), z2, -1.0 / 3.0)
            nc.scalar.copy(shm(7), xz)
            d_, t_ = sc(0), sc(1)
            TT(d_, x2, y2, ALU.subtract)                  # x2-y2
            nc.scalar.copy(shm(8), d_)
            nc.vector.scalar_tensor_tensor(t_, y2, 1.0 / 3.0, x2,
                                           ALU.mult, ALU.subtract)
            TT(shm(9), t_, y, ALU.mult)                   # y*(y2/3-x2)
            TT(shm(10), xy, z, ALU.mult)                  # xyz
            nc.vector.tensor_scalar_add(t_, z2, -0.2)
            TT(shm(11), t_, y, ALU.mult)                  # y*(z2-1/5)
            TT(shm(13), t_, x, ALU.mult)                  # x*(z2-1/5)
            nc.vector.tensor_scalar_add(t_, z2, -0.6)
            TT(shm(12), t_, z, ALU.mult)                  # z*(z2-3/5)
            TT(shm(14), d_, z, ALU.mult)                  # z*(x2-y2)
            nc.vector.scalar_tensor_tensor(t_, y2, 3.0, x2,
                                           ALU.mult, ALU.subtract)
            TT(shm(15), t_, x, ALU.mult)                  # x*(3y2-x2)

            # radial: s_b chain (f32 smalls) + bb staging per b
            C2, SA, SB, TP = sc(0), sc(1), sc(3), sc(4)
            TS(sc(6), sc(2), float(np.pi / CUTOFF), float(-np.pi),
               ALU.mult, ALU.add)
            nc.scalar.activation(sc(7), sc(6), AF.Sin)
            nc.vector.tensor_scalar_mul(SA, sc(7), -1.0)          # s1
            TS(sc(6), sc(2), float(np.pi / CUTOFF), float(-np.pi / 2),
               ALU.mult, ALU.add)
            nc.scalar.activation(sc(7), sc(6), AF.Sin)
            nc.vector.tensor_scalar_mul(C2, sc(7), -2.0)          # 2cos
            for b in range(1, N_BASIS + 1):
                if b == 1:
                    cur = SA
                elif b == 2:
                    TT(SB, C2, SA, ALU.mult)
                    cur = SB
                else:
                    TT(TP, C2, SB if b % 2 else SA, ALU.mult)
                    dst = SA if b % 2 else SB
                    TT(dst, TP, SA if b % 2 else SB, ALU.subtract)
                    cur = dst
                TT(sc(7), cur, sc(5), ALU.mult)
                nc.scalar.copy(BB[:, :, b - 1], sc(7))

            def quarter_chain(q):
                sl = slice(q * Q4, (q + 1) * Q4)
                # radial accumulate: cols 0:12 vector, 12:16 gpsimd
                for b in range(1, N_BASIS + 1):
                    bbv = BB[:, sl, b - 1].unsqueeze(2).to_broadcast(
                        [128, Q4, 12])
                    wbv = WR[:, b - 1:b, 0:12].to_broadcast([128, Q4, 12])
                    bbg = BB[:, sl, b - 1].unsqueeze(2).to_broadcast(
                        [128, Q4, 4])
                    wbg = WR[:, b - 1:b, 12:16].to_broadcast([128, Q4, 4])
                    if b == 1:
                        TT(RR[:, sl, 0:12], bbv, wbv, ALU.mult)
                        GT(RR[:, sl, 12:16], bbg, wbg, ALU.mult)
                    else:
                        TT(TM[:, sl, 0:12], bbv, wbv, ALU.mult)
                        TT(RR[:, sl, 0:12], RR[:, sl, 0:12],
                           TM[:, sl, 0:12], ALU.add)
                        GT(TM[:, sl, 12:16], bbg, wbg, ALU.mult)
                        GT(RR[:, sl, 12:16], RR[:, sl, 12:16],
                           TM[:, sl, 12:16], ALU.add)
                # PT0: l=0 copy (scalar), l=1/2 vector, l=3 gpsimd
                nc.scalar.copy(PT0[:, sl, 0:4], RR[:, sl, 0:4])
                TT(PT0[:, sl, 4:16].rearrange("p k (m n) -> p k m n", n=4),
                   U[:, sl, :].unsqueeze(3).to_broadcast([128, Q4, 3, 4]),
                   RR[:, sl, 4:8].unsqueeze(2).to_broadcast([128, Q4, 3, 4]),
                   ALU.mult)
                TT(PT0[:, sl, 16:36].rearrange("p k (m n) -> p k m n", n=4),
                   SH[:, sl, 0:5].unsqueeze(3).to_broadcast([128, Q4, 5, 4]),
                   RR[:, sl, 8:12].unsqueeze(2).to_broadcast(
                       [128, Q4, 5, 4]),
                   ALU.mult)
                GT(PT0[:, sl, 36:64].rearrange("p k (m n) -> p k m n", n=4),
                   SH[:, sl, 5:12].unsqueeze(3).to_broadcast([128, Q4, 7, 4]),
                   RR[:, sl, 12:16].unsqueeze(2).to_broadcast(
                       [128, Q4, 7, 4]),
                   ALU.mult)
                # block placement: pure DMA into the pre-zeroed slab
                nc.sync.dma_start(PT[0:64, sl, 0:64], PT0[0:64, sl, :])
                nc.sync.dma_start(PT[64:128, sl, 64:128], PT0[64:128, sl, :])
                nc.sync.dma_start(PT0_d[:, sl, :], PT0[:, sl, :])

            phs = []
            for q in range(4):
                quarter_chain(q)
                _emit_scatter(nc, mybir, ppa, PT, GX, AS2,
                              q * Q4, (q + 1) * Q4)
                if q % 2 == 1:
                    h = q // 2
                    ph = pph.tile([32, H2 * CNT_MAX], f32, tag="ph")
                    for c in range(16):
                        nc.tensor.matmul(
                            ph[:], SW[:, c, :],
                            AS2[:, c,
                                h * H2 * CNT_MAX:(h + 1) * H2 * CNT_MAX],
                            start=(c == 0), stop=(c == 15))
                    phs.append(ph)
            # h1 = h1_pre * cemb (vector; emitted last, no pipeline stall)
            for h, ph in enumerate(phs):
                cs = h * H2 * CNT_MAX
                TT(OUTH[:, cs:cs + H2 * CNT_MAX], ph[:],
                   CE[:, cs:cs + H2 * CNT_MAX], ALU.mult)
                nc.sync.dma_start(OUTH_d[:, cs:cs + H2 * CNT_MAX],
                                  OUTH[:, cs:cs + H2 * CNT_MAX])
    nc.compile()
    return nc


def _build_B(KP):
    import concourse.bass as bass
    import concourse.bacc as bacc
    import concourse.tile as tile
    from concourse import mybir

    f32 = mybir.dt.float32
    f16 = mybir.dt.float16
    ALU = mybir.AluOpType
    KPC = KP * CNT_MAX
    H2 = KP // 2
    Q4 = KP // 4

    nc = bacc.Bacc("TRN2", target_bir_lowering=False, debug=False,
                   num_devices=NCORES)
    PT0_d = nc.dram_tensor("pt0", [128, KP, 64], f16,
                           kind="ExternalInput").ap()
    GX_d = nc.dram_tensor("gx", [128, KP, JC], f16,
                          kind="ExternalInput").ap()
    SE_d = nc.dram_tensor("se", [128, 16, 2], f16, kind="ExternalInput").ap()
    OUTE_d = nc.dram_tensor("oute", [2, KPC], f32,
                            kind="ExternalOutput").ap()

    with tile.TileContext(nc) as tc:
        with tc.tile_pool(name="main", bufs=1) as pool, \
             tc.tile_pool(name="pa", bufs=3, space="PSUM") as ppa, \
             tc.tile_pool(name="ph", bufs=2, space="PSUM") as pph:
            GX = pool.tile([128, KP, JC], f16, tag="g")
            SE = pool.tile([128, 16, 2], f16, tag="se")
            PT = pool.tile([128, KP, 128], f16, tag="pt")
            AS2 = pool.tile([128, 16, KPC], f16, tag="as")
            OUTE = pool.tile([2, KPC], f32, tag="oe")

            nc.sync.dma_start(SE[:], SE_d[:])
            for q in range(4):
                sl = slice(q * Q4, (q + 1) * Q4)
                # zero slab quarter (vector/gpsimd split), then DMA blocks
                if q % 2 == 0:
                    nc.vector.memset(PT[:, sl, :], 0.0)
                else:
                    nc.gpsimd.memset(PT[:, sl, :], 0.0)
                nc.sync.dma_start(GX[:, sl], GX_d[:, sl])
                nc.sync.dma_start(PT[0:64, sl, 0:64], PT0_d[0:64, sl, :])
                nc.sync.dma_start(PT[64:128, sl, 64:128],
                                  PT0_d[64:128, sl, :])
                _emit_scatter(nc, mybir, ppa, PT, GX, AS2,
                              q * Q4, (q + 1) * Q4)
                if q % 2 == 1:
                    h = q // 2
                    pe = pph.tile([2, H2 * CNT_MAX], f32, tag="pe")
                    for c in range(16):
                        nc.tensor.matmul(
                            pe[:], SE[:, c, :],
                            AS2[:, c,
                                h * H2 * CNT_MAX:(h + 1) * H2 * CNT_MAX],
                            start=(c == 0), stop=(c == 15))
                    cs = h * H2 * CNT_MAX
                    nc.scalar.copy(OUTE[:, cs:cs + H2 * CNT_MAX], pe[:])
                    nc.sync.dma_start(OUTE_d[:, cs:cs + H2 * CNT_MAX],
                                      OUTE[:, cs:cs + H2 * CNT_MAX])
    nc.compile()
    return nc


def kernel(positions, embed, W_rad, W_inv1, W_inv2, w_out, comp_weights,
           senders, receivers, species, structure_ids):
    from concourse import bass_utils

    positions = np.asarray(positions, np.float32)
    embed = np.asarray(embed, np.float32)
    W_rad = np.asarray(W_rad, np.float32)
    W_inv1 = np.asarray(W_inv1, np.float32)
    W_inv2 = np.asarray(W_inv2, np.float32)
    w_out = np.asarray(w_out, np.float32)
    comp_weights = np.asarray(comp_weights, np.float32)
    senders = np.asarray(senders).astype(np.int64)
    receivers = np.asarray(receivers).astype(np.int64)
    species = np.asarray(species).astype(np.int64)
    structure_ids_np = np.asarray(structure_ids).astype(np.int64)

    KP, tabs = _pack(senders, receivers)
    KPC = KP * CNT_MAX
    if KP not in _prog_cache:
        _prog_cache[KP] = (_build_A(KP), _build_B(KP))
    ncA, ncB = _prog_cache[KP]

    cemb = embed[species]  # [N,16]

    def sw_pack(W):  # [256,16] -> [128, 16, 32] f16
        SW = np.zeros((128, 16, 32), np.float64)
        for b in range(2):
            rows = slice(b * 64, (b + 1) * 64)
            cols = slice(b * 16, (b + 1) * 16)
            for c in range(16):
                SW[rows, c, cols] = ((_SFAC * _TSQ)[:, None] *
                                     W[_LOF * 64 + _NOF * 16 + c, :])
        return SW.astype(np.float16)

    def se_pack(wo):  # [256] -> [128, 16, 2] f16
        SE = np.zeros((128, 16, 2), np.float64)
        for b in range(2):
            for c in range(16):
                SE[b * 64:(b + 1) * 64, c, b] = (
                    _SFAC * _TSQ * wo[_LOF * 64 + _NOF * 16 + c])
        return SE.astype(np.float16)

    SW1 = sw_pack(W_inv1)
    SE2 = se_pack(w_out)
    WRB = np.zeros((8, 16), np.float32)
    for l in range(L_MAX + 1):
        WRB[:, l * 4:(l + 1) * 4] = W_rad[l]
    WRB = np.broadcast_to(WRB[None], (128, 8, 16)).copy()

    def gx_pack(hsrc, tb):
        """GX[p,kp,c*6+j] = h[send[p,kp],c] * MS[p,kp,j], f16."""
        sl, val, MS = tb["slot_send"], tb["slot_val"], tb["MS"]
        hs = hsrc[np.where(val, sl, 0)]
        hs[~val] = 0.0
        gx = hs[:, :, :, None] * MS[:, :, None, :]
        return gx.reshape(128, -1, JC).astype(np.float16)

    PERM = np.array([1, 2, 0])  # (x,y,z) -> (y,z,x)
    maps1 = []
    for core in range(NCORES):
        tb = tabs[core]
        sl, val = tb["slot_send"], tb["slot_val"]
        amap = tb["amap"]
        jidx = tb["MS"].argmax(2)
        bidx = (np.arange(128)[:, None] // 64) * np.ones(
            (1, KP), np.int64)
        ratom = amap[bidx, np.arange(KP)[None, :], jidx]
        ratom = np.where(val, ratom, 0)
        satom = np.where(val, sl, 0)
        pp = np.zeros((128, KP, 6), np.float32)
        pp[:, :, 0:3] = positions[satom][:, :, PERM]
        pp[:, :, 3:6] = positions[ratom][:, :, PERM]
        ce = np.zeros((32, KPC), np.float32)
        av = amap.reshape(2, KPC)
        for b in range(2):
            valid = av[b] >= 0
            ce[b * 16:(b + 1) * 16, valid] = cemb[av[b][valid]].T
        maps1.append(dict(pp=pp, wrb=WRB, gx=gx_pack(cemb, tb),
                          sw=SW1, ce=ce))

    resA = bass_utils.run_bass_kernel_spmd(ncA, maps1,
                                           core_ids=list(range(NCORES)),
                                           trace=PROFILE)
    if PROFILE:
        LAST_PROF.append(resA)

    h1_full = np.zeros((N_ATOMS, C), np.float32)
    for core in range(NCORES):
        amap = tabs[core]["amap"].reshape(2, KPC)
        outh = resA.results[core]["outh"]  # [32, KPC]
        for b in range(2):
            valid = amap[b] >= 0
            h1_full[amap[b][valid]] = outh[b * 16:(b + 1) * 16, valid].T

    maps2 = []
    for core in range(NCORES):
        tb = tabs[core]
        maps2.append(dict(pt0=resA.results[core]["pt0"],
                          gx=gx_pack(h1_full, tb), se=SE2))
    resB = bass_utils.run_bass_kernel_spmd(ncB, maps2,
                                           core_ids=list(range(NCORES)),
                                           trace=PROFILE)
    if PROFILE:
        LAST_PROF.append(resB)

    e_atom = np.zeros(N_ATOMS, np.float32)
    for core in range(NCORES):
        amap = tabs[core]["amap"].reshape(2, KPC)
        oute = resB.results[core]["oute"]  # [2, KPC]
        for b in range(2):
            valid = amap[b] >= 0
            e_atom[amap[b][valid]] = oute[b, valid]
    e_atom += comp_weights[species]
    out = np.zeros(N_STRUCT, np.float32)
    np.add.at(out, structure_ids_np, e_atom)
    return out


# revision 17
# speedup vs baseline: 3.1004x; 1.0813x over previous
import sys
sys.path.insert(0, "/opt/trn_rl_repo")
import numpy as np

N_ATOMS = 10000
N_SPECIES = 8
N_STRUCT = 8
C = 16
N_BASIS = 8
L_MAX = 3
CUTOFF = 5.0
NCORES = 8
NC_AT = N_ATOMS // NCORES
CNT_MAX = 6
JC = CNT_MAX * C  # 96

_prog_cache = {}
PROFILE = False
LAST_PROF = []

# mn row order within a 64-row block: (l, m, n), n fastest
_LOF = np.repeat(np.arange(4), [(2 * l + 1) * 4 for l in range(4)])
_MOF = np.concatenate([np.repeat(np.arange(2 * l + 1), 4) for l in range(4)])
_NOF = np.concatenate([np.tile(np.arange(4), 2 * l + 1) for l in range(4)])
_SFAC = np.repeat([1.0 / np.sqrt(2.0 * l + 1.0) for l in range(4)],
                  [(2 * l + 1) * 4 for l in range(4)]).astype(np.float64)
# device emits raw sh polynomials; true sh = t[m] * raw (sign irrelevant,
# squares only). t^2 folded into SW/SE stationaries host-side.
_T = np.array([0.28209479,
               0.48860251, 0.48860251, 0.48860251,
               1.09254843, 1.09254843, 3 * 0.31539157, 1.09254843,
               0.54627422,
               3 * 0.59004359, 2.89061144, 5 * 0.45704579,
               5 * 0.37317633, 5 * 0.45704579, 1.44530572, 0.59004359],
              np.float64)
_GM = (_LOF * _LOF + _MOF)  # global m index per mn row
_TSQ = (_T[_GM] ** 2).astype(np.float64)


def _pack(senders, receivers):
    """FFD pack receiver atoms into blocks (<=64 edges, <=CNT_MAX atoms);
    pair blocks; block b of a pair owns edge rows [64b, 64b+edges)."""
    recv = np.asarray(receivers).astype(np.int64)
    send = np.asarray(senders).astype(np.int64)
    order = np.argsort(recv, kind="stable")
    ss = send[order]
    deg = np.bincount(recv, minlength=N_ATOMS)
    starts = np.zeros(N_ATOMS + 1, np.int64)
    starts[1:] = np.cumsum(deg)
    core_blocks = []
    for core in range(NCORES):
        a0 = core * NC_AT
        atoms = sorted(range(a0, a0 + NC_AT), key=lambda a: -deg[a])
        blocks = []
        for a in atoms:
            for blk in blocks:
                if blk[0] + deg[a] <= 64 and len(blk[1]) < CNT_MAX:
                    blk[1].append(a); blk[0] += deg[a]; break
            else:
                blocks.append([deg[a], [a]])
        core_blocks.append(blocks)
    KP = max((len(b) + 1) // 2 for b in core_blocks)
    if KP % 4:
        KP += 4 - KP % 4
    tabs = []
    for core in range(NCORES):
        blocks = core_blocks[core]
        slot_send = np.zeros((128, KP), np.int64)
        slot_val = np.zeros((128, KP), bool)
        MS = np.zeros((128, KP, CNT_MAX), np.float32)
        amap = np.full((2, KP, CNT_MAX), -1, np.int64)
        for bi, (ecnt, atoms) in enumerate(blocks):
            kp, b = bi // 2, bi % 2
            row = 64 * b
            for j, a in enumerate(atoms):
                s0, s1 = starts[a], starts[a + 1]
                n = s1 - s0
                slot_send[row:row + n, kp] = ss[s0:s1]
                slot_val[row:row + n, kp] = True
                MS[row:row + n, kp, j] = 1.0
                amap[b, kp, j] = a
                row += n
            assert row <= 64 * b + 64
        tabs.append(dict(slot_send=slot_send, slot_val=slot_val,
                         MS=MS, amap=amap))
    return KP, tabs


def _emit_scatter(nc, mybir, ppa, PT, G, AS2, k0, k1):
    """Scatter matmuls + transposed squares for kp in [k0, k1),
    ragged groups of 4."""
    AF = mybir.ActivationFunctionType
    f32 = mybir.dt.float32
    g0 = k0
    while g0 < k1:
        n = min(4, k1 - g0)
        pa = ppa.tile([128, 4 * JC], f32, tag="pa")
        for q in range(n):
            kp = g0 + q
            nc.tensor.matmul(pa[:, q * JC:(q + 1) * JC],
                             PT[:, kp, :], G[:, kp, :],
                             start=True, stop=True)
        dst = AS2[:, :, g0 * CNT_MAX:(g0 + n) * CNT_MAX].rearrange(
            "p c (k j) -> p k c j", k=n)
        nc.scalar.activation(
            dst, pa[:, 0:n * JC].rearrange("p (k c j) -> p k c j",
                                           k=n, c=16),
            AF.Square)
        g0 += n


def _build_A(KP):
    import concourse.bass as bass
    import concourse.bacc as bacc
    import concourse.tile as tile
    from concourse import mybir

    f32 = mybir.dt.float32
    f16 = mybir.dt.float16
    ALU = mybir.AluOpType
    AF = mybir.ActivationFunctionType
    KPC = KP * CNT_MAX
    H2 = KP // 2
    Q4 = KP // 4

    nc = bacc.Bacc("TRN2", target_bir_lowering=False, debug=False,
                   num_devices=NCORES)
    PP_d = nc.dram_tensor("pp", [128, KP, 6], f32, kind="ExternalInput").ap()
    WR_d = nc.dram_tensor("wrb", [128, 8, 16], f32,
                          kind="ExternalInput").ap()
    GX_d = nc.dram_tensor("gx", [128, KP, JC], f16,
                          kind="ExternalInput").ap()
    SW_d = nc.dram_tensor("sw", [128, 16, 32], f16, kind="ExternalInput").ap()
    CE_d = nc.dram_tensor("ce", [32, KPC], f32, kind="ExternalInput").ap()
    PTZ_d = nc.dram_tensor("ptz", [128, KP, 128], f16,
                           kind="ExternalInput").ap()
    PTF_d = nc.dram_tensor("ptf", [128, KP, 128], f16,
                           kind="ExternalOutput").ap()
    OUTH_d = nc.dram_tensor("outh", [32, KPC], f32,
                            kind="ExternalOutput").ap()

    with tile.TileContext(nc) as tc:
        with tc.tile_pool(name="main", bufs=1) as pool, \
             tc.tile_pool(name="pa", bufs=3, space="PSUM") as ppa, \
             tc.tile_pool(name="ph", bufs=2, space="PSUM") as pph:
            PP = pool.tile([128, KP, 6], f32, tag="pp")
            WR = pool.tile([128, 8, 16], f32, tag="wr")
            GX = pool.tile([128, KP, JC], f16, tag="g")
            SW = pool.tile([128, 16, 32], f16, tag="sw")
            CE = pool.tile([32, KPC], f32, tag="ce")
            nc.sync.dma_start(PP[:], PP_d[:])
            nc.sync.dma_start(WR[:], WR_d[:])
            for q in range(4):
                sl = slice(q * Q4, (q + 1) * Q4)
                nc.sync.dma_start(GX[:, sl], GX_d[:, sl])
            nc.sync.dma_start(SW[:], SW_d[:])
            nc.sync.dma_start(CE[:], CE_d[:])

            RV = pool.tile([128, KP, 3], f32, tag="rv")
            U = pool.tile([128, KP, 3], f32, tag="u")
            SC = pool.tile([128, KP, 12], f32, tag="sc")
            SH = pool.tile([128, KP, 12], f32, tag="sh")  # raw sh m=4..15
            RR = pool.tile([128, KP, 16], f32, tag="rr")
            TM = pool.tile([128, KP, 16], f32, tag="pp2")
            BB = pool.tile([128, KP, 8], f32, tag="bb")
            PT0 = pool.tile([128, KP, 64], f16, tag="pt0")
            PT = pool.tile([128, KP, 128], f16, tag="pt")
            AS2 = pool.tile([128, 16, KPC], f16, tag="as")
            OUTH = pool.tile([32, KPC], f32, tag="oh")

            def sc(i):
                return SC[:, :, i]

            TT = nc.vector.tensor_tensor
            TS = nc.vector.tensor_scalar

            # PT slab arrives pre-zeroed from DRAM (no engine memsets)
            for q in range(4):
                sl = slice(q * Q4, (q + 1) * Q4)
                nc.sync.dma_start(PT[:, sl, :], PTZ_d[:, sl, :])

            # geometry (positions pre-permuted host-side to (y,z,x))
            TT(RV[:], PP[:, :, 3:6], PP[:, :, 0:3], ALU.subtract)
            nc.vector.tensor_mul(U[:], RV[:], RV[:])
            nc.vector.reduce_sum(SC[:, :, 0:1], U[:], mybir.AxisListType.X)
            nc.scalar.activation(sc(1), sc(0), AF.Sqrt)          # r
            nc.vector.tensor_scalar_max(sc(2), sc(1), 1e-6)      # rc
            nc.vector.reciprocal(sc(3), sc(2))                   # rinv
            TT(U[:], RV[:], SC[:, :, 3:4].to_broadcast([128, KP, 3]),
               ALU.mult)
            # fc = 0.5*cos(pi*min(r,5)/5)+0.5 ;  cos(x) = -sin(x - pi/2)
            nc.vector.tensor_scalar_min(sc(6), sc(1), CUTOFF)
            TS(sc(6), sc(6), float(np.pi / CUTOFF), float(-np.pi / 2),
               ALU.mult, ALU.add)
            nc.scalar.activation(sc(7), sc(6), AF.Sin)
            TS(sc(4), sc(7), -0.5, 0.5, ALU.mult, ALU.add)       # fc
            TT(sc(5), sc(4), sc(3), ALU.mult)
            nc.vector.tensor_scalar_mul(sc(5), sc(5),
                                        float(np.sqrt(2.0 / CUTOFF)))  # g
            y, z, x = U[:, :, 0], U[:, :, 1], U[:, :, 2]
            x2, y2, z2, xy, yz, xz = (sc(i) for i in (6, 7, 8, 9, 10, 11))
            nc.vector.tensor_mul(x2, x, x)
            nc.vector.tensor_mul(y2, y, y)
            nc.vector.tensor_mul(z2, z, z)
            nc.vector.tensor_mul(xy, x, y)
            nc.vector.tensor_mul(yz, y, z)
            nc.vector.tensor_mul(xz, x, z)

            # raw sh m=4..15 -> SH cols 0..11
            def shm(m):
                return SH[:, :, m - 4]

            nc.scalar.copy(shm(4), xy)
            nc.scalar.copy(shm(5), yz)
            nc.vector.tensor_scalar_add(shm(6), z2, -1.0 / 3.0)
            nc.scalar.copy(shm(7), xz)
            d_, t_ = sc(0), sc(1)
            TT(d_, x2, y2, ALU.subtract)                  # x2-y2
            nc.scalar.copy(shm(8), d_)
            nc.vector.scalar_tensor_tensor(t_, y2, 1.0 / 3.0, x2,
                                           ALU.mult, ALU.subtract)
            TT(shm(9), t_, y, ALU.mult)                   # y*(y2/3-x2)
            TT(shm(10), xy, z, ALU.mult)                  # xyz
            nc.vector.tensor_scalar_add(t_, z2, -0.2)
            TT(shm(11), t_, y, ALU.mult)                  # y*(z2-1/5)
            TT(shm(13), t_, x, ALU.mult)                  # x*(z2-1/5)
            nc.vector.tensor_scalar_add(t_, z2, -0.6)
            TT(shm(12), t_, z, ALU.mult)                  # z*(z2-3/5)
            TT(shm(14), d_, z, ALU.mult)                  # z*(x2-y2)
            nc.vector.scalar_tensor_tensor(t_, y2, 3.0, x2,
                                           ALU.mult, ALU.subtract)
            TT(shm(15), t_, x, ALU.mult)                  # x*(3y2-x2)

            # radial: s_b chain (f32 smalls) + bb staging per b
            C2, SA, SB, TP = sc(0), sc(1), sc(3), sc(4)
            TS(sc(6), sc(2), float(np.pi / CUTOFF), float(-np.pi),
               ALU.mult, ALU.add)
            nc.scalar.activation(sc(7), sc(6), AF.Sin)
            nc.vector.tensor_scalar_mul(SA, sc(7), -1.0)          # s1
            TS(sc(6), sc(2), float(np.pi / CUTOFF), float(-np.pi / 2),
               ALU.mult, ALU.add)
            nc.scalar.activation(sc(7), sc(6), AF.Sin)
            nc.vector.tensor_scalar_mul(C2, sc(7), -2.0)          # 2cos
            for b in range(1, N_BASIS + 1):
                if b == 1:
                    cur = SA
                elif b == 2:
                    TT(SB, C2, SA, ALU.mult)
                    cur = SB
                else:
                    TT(TP, C2, SB if b % 2 else SA, ALU.mult)
                    dst = SA if b % 2 else SB
                    TT(dst, TP, SA if b % 2 else SB, ALU.subtract)
                    cur = dst
                TT(sc(7), cur, sc(5), ALU.mult)
                nc.scalar.copy(BB[:, :, b - 1], sc(7))

            # radial accumulate: full width, vector only
            for b in range(1, N_BASIS + 1):
                bb = BB[:, :, b - 1].unsqueeze(2).to_broadcast(
                    [128, KP, 16])
                wb = WR[:, b - 1:b, :].to_broadcast([128, KP, 16])
                if b == 1:
                    TT(RR[:], bb, wb, ALU.mult)
                else:
                    TT(TM[:], bb, wb, ALU.mult)
                    TT(RR[:], RR[:], TM[:], ALU.add)

            def quarter_chain(q):
                sl = slice(q * Q4, (q + 1) * Q4)
                # PT0: l=0 copy (scalar), l=1/2/3 vector
                nc.scalar.copy(PT0[:, sl, 0:4], RR[:, sl, 0:4])
                TT(PT0[:, sl, 4:16].rearrange("p k (m n) -> p k m n", n=4),
                   U[:, sl, :].unsqueeze(3).to_broadcast([128, Q4, 3, 4]),
                   RR[:, sl, 4:8].unsqueeze(2).to_broadcast([128, Q4, 3, 4]),
                   ALU.mult)
                TT(PT0[:, sl, 16:36].rearrange("p k (m n) -> p k m n", n=4),
                   SH[:, sl, 0:5].unsqueeze(3).to_broadcast([128, Q4, 5, 4]),
                   RR[:, sl, 8:12].unsqueeze(2).to_broadcast(
                       [128, Q4, 5, 4]),
                   ALU.mult)
                TT(PT0[:, sl, 36:64].rearrange("p k (m n) -> p k m n", n=4),
                   SH[:, sl, 5:12].unsqueeze(3).to_broadcast([128, Q4, 7, 4]),
                   RR[:, sl, 12:16].unsqueeze(2).to_broadcast(
                       [128, Q4, 7, 4]),
                   ALU.mult)
                # block placement: pure DMA into the pre-zeroed slab
                nc.sync.dma_start(PT[0:64, sl, 0:64], PT0[0:64, sl, :])
                nc.sync.dma_start(PT[64:128, sl, 64:128], PT0[64:128, sl, :])
                nc.sync.dma_start(PTF_d[:, sl, :], PT[:, sl, :])

            phs = []
            for q in range(4):
                quarter_chain(q)
                _emit_scatter(nc, mybir, ppa, PT, GX, AS2,
                              q * Q4, (q + 1) * Q4)
                if q % 2 == 1:
                    h = q // 2
                    ph = pph.tile([32, H2 * CNT_MAX], f32, tag="ph")
                    for c in range(16):
                        nc.tensor.matmul(
                            ph[:], SW[:, c, :],
                            AS2[:, c,
                                h * H2 * CNT_MAX:(h + 1) * H2 * CNT_MAX],
                            start=(c == 0), stop=(c == 15))
                    phs.append(ph)
            # h1 = h1_pre * cemb (vector; emitted last, no pipeline stall)
            for h, ph in enumerate(phs):
                cs = h * H2 * CNT_MAX
                TT(OUTH[:, cs:cs + H2 * CNT_MAX], ph[:],
                   CE[:, cs:cs + H2 * CNT_MAX], ALU.mult)
                nc.sync.dma_start(OUTH_d[:, cs:cs + H2 * CNT_MAX],
                                  OUTH[:, cs:cs + H2 * CNT_MAX])
    nc.compile()
    return nc


def _build_B(KP):
    import concourse.bass as bass
    import concourse.bacc as bacc
    import concourse.tile as tile
    from concourse import mybir

    f32 = mybir.dt.float32
    f16 = mybir.dt.float16
    ALU = mybir.AluOpType
    KPC = KP * CNT_MAX
    H2 = KP // 2
    Q4 = KP // 4

    nc = bacc.Bacc("TRN2", target_bir_lowering=False, debug=False,
                   num_devices=NCORES)
    PTF_d = nc.dram_tensor("ptf", [128, KP, 128], f16,
                           kind="ExternalInput").ap()
    GX_d = nc.dram_tensor("gx", [128, KP, JC], f16,
                          kind="ExternalInput").ap()
    SE_d = nc.dram_tensor("se", [128, 16, 2], f16, kind="ExternalInput").ap()
    OUTE_d = nc.dram_tensor("oute", [2, KPC], f32,
                            kind="ExternalOutput").ap()

    with tile.TileContext(nc) as tc:
        with tc.tile_pool(name="main", bufs=1) as pool, \
             tc.tile_pool(name="pa", bufs=3, space="PSUM") as ppa, \
             tc.tile_pool(name="ph", bufs=2, space="PSUM") as pph:
            GX = pool.tile([128, KP, JC], f16, tag="g")
            SE = pool.tile([128, 16, 2], f16, tag="se")
            PT = pool.tile([128, KP, 128], f16, tag="pt")
            AS2 = pool.tile([128, 16, KPC], f16, tag="as")
            OUTE = pool.tile([2, KPC], f32, tag="oe")

            nc.sync.dma_start(SE[:], SE_d[:])
            for q in range(4):
                sl = slice(q * Q4, (q + 1) * Q4)
                nc.sync.dma_start(GX[:, sl], GX_d[:, sl])
                nc.sync.dma_start(PT[:, sl, :], PTF_d[:, sl, :])
                _emit_scatter(nc, mybir, ppa, PT, GX, AS2,
                              q * Q4, (q + 1) * Q4)
                if q % 2 == 1:
                    h = q // 2
                    pe = pph.tile([2, H2 * CNT_MAX], f32, tag="pe")
                    for c in range(16):
                        nc.tensor.matmul(
                            pe[:], SE[:, c, :],
                            AS2[:, c,
                                h * H2 * CNT_MAX:(h + 1) * H2 * CNT_MAX],
                            start=(c == 0), stop=(c == 15))
                    cs = h * H2 * CNT_MAX
                    nc.scalar.copy(OUTE[:, cs:cs + H2 * CNT_MAX], pe[:])
                    nc.sync.dma_start(OUTE_d[:, cs:cs + H2 * CNT_MAX],
                                      OUTE[:, cs:cs + H2 * CNT_MAX])
    nc.compile()
    return nc


def kernel(positions, embed, W_rad, W_inv1, W_inv2, w_out, comp_weights,
           senders, receivers, species, structure_ids):
    from concourse import bass_utils

    positions = np.asarray(positions, np.float32)
    embed = np.asarray(embed, np.float32)
    W_rad = np.asarray(W_rad, np.float32)
    W_inv1 = np.asarray(W_inv1, np.float32)
    W_inv2 = np.asarray(W_inv2, np.float32)
    w_out = np.asarray(w_out, np.float32)
    comp_weights = np.asarray(comp_weights, np.float32)
    senders = np.asarray(senders).astype(np.int64)
    receivers = np.asarray(receivers).astype(np.int64)
    species = np.asarray(species).astype(np.int64)
    structure_ids_np = np.asarray(structure_ids).astype(np.int64)

    KP, tabs = _pack(senders, receivers)
    KPC = KP * CNT_MAX
    if KP not in _prog_cache:
        _prog_cache[KP] = (_build_A(KP), _build_B(KP))
    ncA, ncB = _prog_cache[KP]

    cemb = embed[species]  # [N,16]

    def sw_pack(W):  # [256,16] -> [128, 16, 32] f16
        SW = np.zeros((128, 16, 32), np.float64)
        for b in range(2):
            rows = slice(b * 64, (b + 1) * 64)
            cols = slice(b * 16, (b + 1) * 16)
            for c in range(16):
                SW[rows, c, cols] = ((_SFAC * _TSQ)[:, None] *
                                     W[_LOF * 64 + _NOF * 16 + c, :])
        return SW.astype(np.float16)

    def se_pack(wo):  # [256] -> [128, 16, 2] f16
        SE = np.zeros((128, 16, 2), np.float64)
        for b in range(2):
            for c in range(16):
                SE[b * 64:(b + 1) * 64, c, b] = (
                    _SFAC * _TSQ * wo[_LOF * 64 + _NOF * 16 + c])
        return SE.astype(np.float16)

    SW1 = sw_pack(W_inv1)
    SE2 = se_pack(w_out)
    WRB = np.zeros((8, 16), np.float32)
    for l in range(L_MAX + 1):
        WRB[:, l * 4:(l + 1) * 4] = W_rad[l]
    WRB = np.broadcast_to(WRB[None], (128, 8, 16)).copy()

    def gx_pack(hsrc, tb):
        """GX[p,kp,c*6+j] = h[send[p,kp],c] * MS[p,kp,j], f16."""
        sl, val, MS = tb["slot_send"], tb["slot_val"], tb["MS"]
        hs = hsrc[np.where(val, sl, 0)]
        hs[~val] = 0.0
        gx = hs[:, :, :, None] * MS[:, :, None, :]
        return gx.reshape(128, -1, JC).astype(np.float16)

    PERM = np.array([1, 2, 0])  # (x,y,z) -> (y,z,x)
    PTZ = np.zeros((128, KP, 128), np.float16)
    maps1 = []
    for core in range(NCORES):
        tb = tabs[core]
        sl, val = tb["slot_send"], tb["slot_val"]
        amap = tb["amap"]
        jidx = tb["MS"].argmax(2)
        bidx = (np.arange(128)[:, None] // 64) * np.ones(
            (1, KP), np.int64)
        ratom = amap[bidx, np.arange(KP)[None, :], jidx]
        ratom = np.where(val, ratom, 0)
        satom = np.where(val, sl, 0)
        pp = np.zeros((128, KP, 6), np.float32)
        pp[:, :, 0:3] = positions[satom][:, :, PERM]
        pp[:, :, 3:6] = positions[ratom][:, :, PERM]
        ce = np.zeros((32, KPC), np.float32)
        av = amap.reshape(2, KPC)
        for b in range(2):
            valid = av[b] >= 0
            ce[b * 16:(b + 1) * 16, valid] = cemb[av[b][valid]].T
        maps1.append(dict(pp=pp, wrb=WRB, gx=gx_pack(cemb, tb),
                          sw=SW1, ce=ce, ptz=PTZ))

    resA = bass_utils.run_bass_kernel_spmd(ncA, maps1,
                                           core_ids=list(range(NCORES)),
                                           trace=PROFILE)
    if PROFILE:
        LAST_PROF.append(resA)

    h1_full = np.zeros((N_ATOMS, C), np.float32)
    for core in range(NCORES):
        amap = tabs[core]["amap"].reshape(2, KPC)
        outh = resA.results[core]["outh"]  # [32, KPC]
        for b in range(2):
            valid = amap[b] >= 0
            h1_full[amap[b][valid]] = outh[b * 16:(b + 1) * 16, valid].T

    maps2 = []
    for core in range(NCORES):
        tb = tabs[core]
        maps2.append(dict(ptf=resA.results[core]["ptf"],
                          gx=gx_pack(h1_full, tb), se=SE2))
    resB = bass_utils.run_bass_kernel_spmd(ncB, maps2,
                                           core_ids=list(range(NCORES)),
                                           trace=PROFILE)
    if PROFILE:
        LAST_PROF.append(resB)

    e_atom = np.zeros(N_ATOMS, np.float32)
    for core in range(NCORES):
        amap = tabs[core]["amap"].reshape(2, KPC)
        oute = resB.results[core]["oute"]  # [2, KPC]
        for b in range(2):
            valid = amap[b] >= 0
            e_atom[amap[b][valid]] = oute[b, valid]
    e_atom += comp_weights[species]
    out = np.zeros(N_STRUCT, np.float32)
    np.add.at(out, structure_ids_np, e_atom)
    return out


# revision 18
# speedup vs baseline: 3.3477x; 1.0798x over previous
import sys
sys.path.insert(0, "/opt/trn_rl_repo")
import numpy as np

N_ATOMS = 10000
N_SPECIES = 8
N_STRUCT = 8
C = 16
N_BASIS = 8
L_MAX = 3
CUTOFF = 5.0
NCORES = 8
NC_AT = N_ATOMS // NCORES
CNT_MAX = 6
JC = CNT_MAX * C  # 96

_prog_cache = {}
PROFILE = False
LAST_PROF = []

# mn row order within a 64-row block: (l, m, n), n fastest
_LOF = np.repeat(np.arange(4), [(2 * l + 1) * 4 for l in range(4)])
_MOF = np.concatenate([np.repeat(np.arange(2 * l + 1), 4) for l in range(4)])
_NOF = np.concatenate([np.tile(np.arange(4), 2 * l + 1) for l in range(4)])
_SFAC = np.repeat([1.0 / np.sqrt(2.0 * l + 1.0) for l in range(4)],
                  [(2 * l + 1) * 4 for l in range(4)]).astype(np.float64)
# device emits raw sh polynomials; true sh = t[m] * raw (sign irrelevant,
# squares only). t^2 folded into SW/SE stationaries host-side.
_T = np.array([0.28209479,
               0.48860251, 0.48860251, 0.48860251,
               1.09254843, 1.09254843, 3 * 0.31539157, 1.09254843,
               0.54627422,
               3 * 0.59004359, 2.89061144, 5 * 0.45704579,
               5 * 0.37317633, 5 * 0.45704579, 1.44530572, 0.59004359],
              np.float64)
_GM = (_LOF * _LOF + _MOF)  # global m index per mn row
_TSQ = (_T[_GM] ** 2).astype(np.float64)


def _pack(senders, receivers):
    """FFD pack receiver atoms into blocks (<=64 edges, <=CNT_MAX atoms);
    pair blocks; block b of a pair owns edge rows [64b, 64b+edges)."""
    recv = np.asarray(receivers).astype(np.int64)
    send = np.asarray(senders).astype(np.int64)
    order = np.argsort(recv, kind="stable")
    ss = send[order]
    deg = np.bincount(recv, minlength=N_ATOMS)
    starts = np.zeros(N_ATOMS + 1, np.int64)
    starts[1:] = np.cumsum(deg)
    core_blocks = []
    for core in range(NCORES):
        a0 = core * NC_AT
        atoms = sorted(range(a0, a0 + NC_AT), key=lambda a: -deg[a])
        blocks = []
        for a in atoms:
            for blk in blocks:
                if blk[0] + deg[a] <= 64 and len(blk[1]) < CNT_MAX:
                    blk[1].append(a); blk[0] += deg[a]; break
            else:
                blocks.append([deg[a], [a]])
        core_blocks.append(blocks)
    KP = max((len(b) + 1) // 2 for b in core_blocks)
    if KP % 4:
        KP += 4 - KP % 4
    tabs = []
    for core in range(NCORES):
        blocks = core_blocks[core]
        slot_send = np.zeros((128, KP), np.int64)
        slot_val = np.zeros((128, KP), bool)
        MS = np.zeros((128, KP, CNT_MAX), np.float32)
        amap = np.full((2, KP, CNT_MAX), -1, np.int64)
        for bi, (ecnt, atoms) in enumerate(blocks):
            kp, b = bi // 2, bi % 2
            row = 64 * b
            for j, a in enumerate(atoms):
                s0, s1 = starts[a], starts[a + 1]
                n = s1 - s0
                slot_send[row:row + n, kp] = ss[s0:s1]
                slot_val[row:row + n, kp] = True
                MS[row:row + n, kp, j] = 1.0
                amap[b, kp, j] = a
                row += n
            assert row <= 64 * b + 64
        tabs.append(dict(slot_send=slot_send, slot_val=slot_val,
                         MS=MS, amap=amap))
    return KP, tabs


def _emit_scatter(nc, mybir, ppa, PT, G, AS2, k0, k1):
    """Scatter matmuls + transposed squares for kp in [k0, k1),
    ragged groups of 4."""
    AF = mybir.ActivationFunctionType
    f32 = mybir.dt.float32
    g0 = k0
    while g0 < k1:
        n = min(4, k1 - g0)
        pa = ppa.tile([128, 4 * JC], f32, tag="pa")
        for q in range(n):
            kp = g0 + q
            nc.tensor.matmul(pa[:, q * JC:(q + 1) * JC],
                             PT[:, kp, :], G[:, kp, :],
                             start=True, stop=True)
        dst = AS2[:, :, g0 * CNT_MAX:(g0 + n) * CNT_MAX].rearrange(
            "p c (k j) -> p k c j", k=n)
        nc.scalar.activation(
            dst, pa[:, 0:n * JC].rearrange("p (k c j) -> p k c j",
                                           k=n, c=16),
            AF.Square)
        g0 += n


def _build_A(KP):
    import concourse.bass as bass
    import concourse.bacc as bacc
    import concourse.tile as tile
    from concourse import mybir

    f32 = mybir.dt.float32
    f16 = mybir.dt.float16
    ALU = mybir.AluOpType
    AF = mybir.ActivationFunctionType
    KPC = KP * CNT_MAX
    H2 = KP // 2
    Q4 = KP // 4

    nc = bacc.Bacc("TRN2", target_bir_lowering=False, debug=False,
                   num_devices=NCORES)
    PP_d = nc.dram_tensor("pp", [128, KP, 6], f32, kind="ExternalInput").ap()
    WR_d = nc.dram_tensor("wrb", [128, 8, 16], f32,
                          kind="ExternalInput").ap()
    GX_d = nc.dram_tensor("gx", [128, KP, JC], f16,
                          kind="ExternalInput").ap()
    SW_d = nc.dram_tensor("sw", [128, 16, 32], f16, kind="ExternalInput").ap()
    CE_d = nc.dram_tensor("ce", [32, KPC], f32, kind="ExternalInput").ap()
    PTZ_d = nc.dram_tensor("ptz", [128, KP, 128], f16,
                           kind="ExternalInput").ap()
    PTF_d = nc.dram_tensor("ptf", [128, KP, 128], f16,
                           kind="ExternalOutput").ap()
    OUTH_d = nc.dram_tensor("outh", [32, KPC], f32,
                            kind="ExternalOutput").ap()

    with tile.TileContext(nc) as tc:
        with tc.tile_pool(name="main", bufs=1) as pool, \
             tc.tile_pool(name="pa", bufs=3, space="PSUM") as ppa, \
             tc.tile_pool(name="ph", bufs=2, space="PSUM") as pph:
            PP = pool.tile([128, KP, 6], f32, tag="pp")
            WR = pool.tile([128, 8, 16], f32, tag="wr")
            GX = pool.tile([128, KP, JC], f16, tag="g")
            SW = pool.tile([128, 16, 32], f16, tag="sw")
            CE = pool.tile([32, KPC], f32, tag="ce")
            nc.sync.dma_start(PP[:], PP_d[:])
            nc.sync.dma_start(WR[:], WR_d[:])
            for q in range(4):
                sl = slice(q * Q4, (q + 1) * Q4)
                nc.sync.dma_start(GX[:, sl], GX_d[:, sl])
            nc.sync.dma_start(SW[:], SW_d[:])
            nc.sync.dma_start(CE[:], CE_d[:])

            RV = pool.tile([128, KP, 3], f32, tag="rv")
            U = pool.tile([128, KP, 3], f32, tag="u")
            SC = pool.tile([128, KP, 12], f32, tag="sc")
            SH = pool.tile([128, KP, 12], f32, tag="sh")  # raw sh m=4..15
            RR = pool.tile([128, KP, 16], f32, tag="rr")
            TM = pool.tile([128, KP, 16], f32, tag="pp2")
            RG0 = pool.tile([128, KP, 4], f32, tag="rg0")
            PT = pool.tile([128, KP, 128], f16, tag="pt")
            AS2 = pool.tile([128, 16, KPC], f16, tag="as")
            OUTH = pool.tile([32, KPC], f32, tag="oh")

            def sc(i):
                return SC[:, :, i]

            TT = nc.vector.tensor_tensor
            TS = nc.vector.tensor_scalar

            # PT slab arrives pre-zeroed from DRAM (no engine memsets)
            for q in range(4):
                sl = slice(q * Q4, (q + 1) * Q4)
                nc.sync.dma_start(PT[:, sl, :], PTZ_d[:, sl, :])

            # geometry (positions pre-permuted host-side to (y,z,x))
            TT(RV[:], PP[:, :, 3:6], PP[:, :, 0:3], ALU.subtract)
            nc.vector.tensor_mul(U[:], RV[:], RV[:])
            nc.vector.reduce_sum(SC[:, :, 0:1], U[:], mybir.AxisListType.X)
            nc.scalar.activation(sc(1), sc(0), AF.Sqrt)          # r
            nc.vector.tensor_scalar_max(sc(2), sc(1), 1e-6)      # rc
            nc.vector.reciprocal(sc(3), sc(2))                   # rinv
            TT(U[:], RV[:], SC[:, :, 3:4].to_broadcast([128, KP, 3]),
               ALU.mult)
            # fc = 0.5*cos(pi*min(r,5)/5)+0.5 ;  cos(x) = -sin(x - pi/2)
            nc.vector.tensor_scalar_min(sc(6), sc(1), CUTOFF)
            TS(sc(6), sc(6), float(np.pi / CUTOFF), float(-np.pi / 2),
               ALU.mult, ALU.add)
            nc.scalar.activation(sc(7), sc(6), AF.Sin)
            TS(sc(4), sc(7), -0.5, 0.5, ALU.mult, ALU.add)       # fc
            TT(sc(5), sc(4), sc(3), ALU.mult)
            nc.vector.tensor_scalar_mul(sc(5), sc(5),
                                        float(np.sqrt(2.0 / CUTOFF)))  # g
            y, z, x = U[:, :, 0], U[:, :, 1], U[:, :, 2]
            x2, y2, z2, xy, yz, xz = (sc(i) for i in (6, 7, 8, 9, 10, 11))
            nc.vector.tensor_mul(x2, x, x)
            nc.vector.tensor_mul(y2, y, y)
            nc.vector.tensor_mul(z2, z, z)
            nc.vector.tensor_mul(xy, x, y)
            nc.vector.tensor_mul(yz, y, z)
            nc.vector.tensor_mul(xz, x, z)

            # raw sh m=4..15 -> SH cols 0..11
            def shm(m):
                return SH[:, :, m - 4]

            nc.scalar.copy(shm(4), xy)
            nc.scalar.copy(shm(5), yz)
            nc.vector.tensor_scalar_add(shm(6), z2, -1.0 / 3.0)
            nc.scalar.copy(shm(7), xz)
            d_, t_ = sc(0), sc(1)
            TT(d_, x2, y2, ALU.subtract)                  # x2-y2
            nc.scalar.copy(shm(8), d_)
            nc.vector.scalar_tensor_tensor(t_, y2, 1.0 / 3.0, x2,
                                           ALU.mult, ALU.subtract)
            TT(shm(9), t_, y, ALU.mult)                   # y*(y2/3-x2)
            TT(shm(10), xy, z, ALU.mult)                  # xyz
            nc.vector.tensor_scalar_add(t_, z2, -0.2)
            TT(shm(11), t_, y, ALU.mult)                  # y*(z2-1/5)
            TT(shm(13), t_, x, ALU.mult)                  # x*(z2-1/5)
            nc.vector.tensor_scalar_add(t_, z2, -0.6)
            TT(shm(12), t_, z, ALU.mult)                  # z*(z2-3/5)
            TT(shm(14), d_, z, ALU.mult)                  # z*(x2-y2)
            nc.vector.scalar_tensor_tensor(t_, y2, 3.0, x2,
                                           ALU.mult, ALU.subtract)
            TT(shm(15), t_, x, ALU.mult)                  # x*(3y2-x2)
            # fold g into SH and U (radial then accumulates raw sin terms)
            gb12 = SC[:, :, 5:6].to_broadcast([128, KP, 12])
            TT(SH[:], SH[:], gb12, ALU.mult)
            TT(U[:], U[:], SC[:, :, 5:6].to_broadcast([128, KP, 3]),
               ALU.mult)

            # radial: s_b recurrence with inline accumulate (raw, no g)
            C2, SA, SB, TP = sc(0), sc(1), sc(3), sc(4)
            TS(sc(6), sc(2), float(np.pi / CUTOFF), float(-np.pi),
               ALU.mult, ALU.add)
            nc.scalar.activation(sc(7), sc(6), AF.Sin)
            nc.vector.tensor_scalar_mul(SA, sc(7), -1.0)          # s1
            TS(sc(6), sc(2), float(np.pi / CUTOFF), float(-np.pi / 2),
               ALU.mult, ALU.add)
            nc.scalar.activation(sc(7), sc(6), AF.Sin)
            nc.vector.tensor_scalar_mul(C2, sc(7), -2.0)          # 2cos
            for b in range(1, N_BASIS + 1):
                if b == 1:
                    cur = SA
                elif b == 2:
                    TT(SB, C2, SA, ALU.mult)
                    cur = SB
                else:
                    TT(TP, C2, SB if b % 2 else SA, ALU.mult)
                    dst = SA if b % 2 else SB
                    TT(dst, TP, SA if b % 2 else SB, ALU.subtract)
                    cur = dst
                bb = cur.unsqueeze(2).to_broadcast([128, KP, 16])
                wb = WR[:, b - 1:b, :].to_broadcast([128, KP, 16])
                if b == 1:
                    TT(RR[:], bb, wb, ALU.mult)
                else:
                    TT(TM[:], bb, wb, ALU.mult)
                    TT(RR[:], RR[:], TM[:], ALU.add)
            # l0 needs g*RR (g folded into SH/U covers l1..l3 only)
            TT(RG0[:], RR[:, :, 0:4],
               SC[:, :, 5:6].to_broadcast([128, KP, 4]), ALU.mult)

            def quarter_chain(q):
                sl = slice(q * Q4, (q + 1) * Q4)
                # write PT diagonal blocks directly (partition-split)
                for hp in range(2):
                    pr = slice(hp * 64, (hp + 1) * 64)
                    co = hp * 64
                    nc.scalar.copy(PT[pr, sl, co:co + 4], RG0[pr, sl, :])
                    TT(PT[pr, sl, co + 4:co + 16].rearrange(
                        "p k (m n) -> p k m n", n=4),
                       U[pr, sl, :].unsqueeze(3).to_broadcast(
                           [64, Q4, 3, 4]),
                       RR[pr, sl, 4:8].unsqueeze(2).to_broadcast(
                           [64, Q4, 3, 4]),
                       ALU.mult)
                    TT(PT[pr, sl, co + 16:co + 36].rearrange(
                        "p k (m n) -> p k m n", n=4),
                       SH[pr, sl, 0:5].unsqueeze(3).to_broadcast(
                           [64, Q4, 5, 4]),
                       RR[pr, sl, 8:12].unsqueeze(2).to_broadcast(
                           [64, Q4, 5, 4]),
                       ALU.mult)
                    TT(PT[pr, sl, co + 36:co + 64].rearrange(
                        "p k (m n) -> p k m n", n=4),
                       SH[pr, sl, 5:12].unsqueeze(3).to_broadcast(
                           [64, Q4, 7, 4]),
                       RR[pr, sl, 12:16].unsqueeze(2).to_broadcast(
                           [64, Q4, 7, 4]),
                       ALU.mult)
                nc.sync.dma_start(PTF_d[:, sl, :], PT[:, sl, :])

            phs = []
            for q in range(4):
                quarter_chain(q)
                _emit_scatter(nc, mybir, ppa, PT, GX, AS2,
                              q * Q4, (q + 1) * Q4)
                if q % 2 == 1:
                    h = q // 2
                    ph = pph.tile([32, H2 * CNT_MAX], f32, tag="ph")
                    for c in range(16):
                        nc.tensor.matmul(
                            ph[:], SW[:, c, :],
                            AS2[:, c,
                                h * H2 * CNT_MAX:(h + 1) * H2 * CNT_MAX],
                            start=(c == 0), stop=(c == 15))
                    phs.append(ph)
            # h1 = h1_pre * cemb (vector; emitted last, no pipeline stall)
            for h, ph in enumerate(phs):
                cs = h * H2 * CNT_MAX
                TT(OUTH[:, cs:cs + H2 * CNT_MAX], ph[:],
                   CE[:, cs:cs + H2 * CNT_MAX], ALU.mult)
                nc.sync.dma_start(OUTH_d[:, cs:cs + H2 * CNT_MAX],
                                  OUTH[:, cs:cs + H2 * CNT_MAX])
    nc.compile()
    return nc


def _build_B(KP):
    import concourse.bass as bass
    import concourse.bacc as bacc
    import concourse.tile as tile
    from concourse import mybir

    f32 = mybir.dt.float32
    f16 = mybir.dt.float16
    ALU = mybir.AluOpType
    KPC = KP * CNT_MAX
    H2 = KP // 2
    Q4 = KP // 4

    nc = bacc.Bacc("TRN2", target_bir_lowering=False, debug=False,
                   num_devices=NCORES)
    PTF_d = nc.dram_tensor("ptf", [128, KP, 128], f16,
                           kind="ExternalInput").ap()
    GX_d = nc.dram_tensor("gx", [128, KP, JC], f16,
                          kind="ExternalInput").ap()
    SE_d = nc.dram_tensor("se", [128, 16, 2], f16, kind="ExternalInput").ap()
    OUTE_d = nc.dram_tensor("oute", [2, KPC], f32,
                            kind="ExternalOutput").ap()

    with tile.TileContext(nc) as tc:
        with tc.tile_pool(name="main", bufs=1) as pool, \
             tc.tile_pool(name="pa", bufs=3, space="PSUM") as ppa, \
             tc.tile_pool(name="ph", bufs=2, space="PSUM") as pph:
            GX = pool.tile([128, KP, JC], f16, tag="g")
            SE = pool.tile([128, 16, 2], f16, tag="se")
            PT = pool.tile([128, KP, 128], f16, tag="pt")
            AS2 = pool.tile([128, 16, KPC], f16, tag="as")
            OUTE = pool.tile([2, KPC], f32, tag="oe")

            nc.sync.dma_start(SE[:], SE_d[:])
            for q in range(4):
                sl = slice(q * Q4, (q + 1) * Q4)
                nc.sync.dma_start(GX[:, sl], GX_d[:, sl])
                nc.sync.dma_start(PT[:, sl, :], PTF_d[:, sl, :])
                _emit_scatter(nc, mybir, ppa, PT, GX, AS2,
                              q * Q4, (q + 1) * Q4)
                if q % 2 == 1:
                    h = q // 2
                    pe = pph.tile([2, H2 * CNT_MAX], f32, tag="pe")
                    for c in range(16):
                        nc.tensor.matmul(
                            pe[:], SE[:, c, :],
                            AS2[:, c,
                                h * H2 * CNT_MAX:(h + 1) * H2 * CNT_MAX],
                            start=(c == 0), stop=(c == 15))
                    cs = h * H2 * CNT_MAX
                    nc.scalar.copy(OUTE[:, cs:cs + H2 * CNT_MAX], pe[:])
                    nc.sync.dma_start(OUTE_d[:, cs:cs + H2 * CNT_MAX],
                                      OUTE[:, cs:cs + H2 * CNT_MAX])
    nc.compile()
    return nc


def kernel(positions, embed, W_rad, W_inv1, W_inv2, w_out, comp_weights,
           senders, receivers, species, structure_ids):
    from concourse import bass_utils

    positions = np.asarray(positions, np.float32)
    embed = np.asarray(embed, np.float32)
    W_rad = np.asarray(W_rad, np.float32)
    W_inv1 = np.asarray(W_inv1, np.float32)
    W_inv2 = np.asarray(W_inv2, np.float32)
    w_out = np.asarray(w_out, np.float32)
    comp_weights = np.asarray(comp_weights, np.float32)
    senders = np.asarray(senders).astype(np.int64)
    receivers = np.asarray(receivers).astype(np.int64)
    species = np.asarray(species).astype(np.int64)
    structure_ids_np = np.asarray(structure_ids).astype(np.int64)

    KP, tabs = _pack(senders, receivers)
    KPC = KP * CNT_MAX
    if KP not in _prog_cache:
        _prog_cache[KP] = (_build_A(KP), _build_B(KP))
    ncA, ncB = _prog_cache[KP]

    cemb = embed[species]  # [N,16]

    def sw_pack(W):  # [256,16] -> [128, 16, 32] f16
        SW = np.zeros((128, 16, 32), np.float64)
        for b in range(2):
            rows = slice(b * 64, (b + 1) * 64)
            cols = slice(b * 16, (b + 1) * 16)
            for c in range(16):
                SW[rows, c, cols] = ((_SFAC * _TSQ)[:, None] *
                                     W[_LOF * 64 + _NOF * 16 + c, :])
        return SW.astype(np.float16)

    def se_pack(wo):  # [256] -> [128, 16, 2] f16
        SE = np.zeros((128, 16, 2), np.float64)
        for b in range(2):
            for c in range(16):
                SE[b * 64:(b + 1) * 64, c, b] = (
                    _SFAC * _TSQ * wo[_LOF * 64 + _NOF * 16 + c])
        return SE.astype(np.float16)

    SW1 = sw_pack(W_inv1)
    SE2 = se_pack(w_out)
    WRB = np.zeros((8, 16), np.float32)
    for l in range(L_MAX + 1):
        WRB[:, l * 4:(l + 1) * 4] = W_rad[l]
    WRB = np.broadcast_to(WRB[None], (128, 8, 16)).copy()

    def gx_pack(hsrc, tb):
        """GX[p,kp,c*6+j] = h[send[p,kp],c] * MS[p,kp,j], f16."""
        sl, val, MS = tb["slot_send"], tb["slot_val"], tb["MS"]
        hs = hsrc[np.where(val, sl, 0)]
        hs[~val] = 0.0
        gx = hs[:, :, :, None] * MS[:, :, None, :]
        return gx.reshape(128, -1, JC).astype(np.float16)

    PERM = np.array([1, 2, 0])  # (x,y,z) -> (y,z,x)
    PTZ = np.zeros((128, KP, 128), np.float16)
    maps1 = []
    for core in range(NCORES):
        tb = tabs[core]
        sl, val = tb["slot_send"], tb["slot_val"]
        amap = tb["amap"]
        jidx = tb["MS"].argmax(2)
        bidx = (np.arange(128)[:, None] // 64) * np.ones(
            (1, KP), np.int64)
        ratom = amap[bidx, np.arange(KP)[None, :], jidx]
        ratom = np.where(val, ratom, 0)
        satom = np.where(val, sl, 0)
        pp = np.zeros((128, KP, 6), np.float32)
        pp[:, :, 0:3] = positions[satom][:, :, PERM]
        pp[:, :, 3:6] = positions[ratom][:, :, PERM]
        ce = np.zeros((32, KPC), np.float32)
        av = amap.reshape(2, KPC)
        for b in range(2):
            valid = av[b] >= 0
            ce[b * 16:(b + 1) * 16, valid] = cemb[av[b][valid]].T
        maps1.append(dict(pp=pp, wrb=WRB, gx=gx_pack(cemb, tb),
                          sw=SW1, ce=ce, ptz=PTZ))

    resA = bass_utils.run_bass_kernel_spmd(ncA, maps1,
                                           core_ids=list(range(NCORES)),
                                           trace=PROFILE)
    if PROFILE:
        LAST_PROF.append(resA)

    h1_full = np.zeros((N_ATOMS, C), np.float32)
    for core in range(NCORES):
        amap = tabs[core]["amap"].reshape(2, KPC)
        outh = resA.results[core]["outh"]  # [32, KPC]
        for b in range(2):
            valid = amap[b] >= 0
            h1_full[amap[b][valid]] = outh[b * 16:(b + 1) * 16, valid].T

    maps2 = []
    for core in range(NCORES):
        tb = tabs[core]
        maps2.append(dict(ptf=resA.results[core]["ptf"],
                          gx=gx_pack(h1_full, tb), se=SE2))
    resB = bass_utils.run_bass_kernel_spmd(ncB, maps2,
                                           core_ids=list(range(NCORES)),
                                           trace=PROFILE)
    if PROFILE:
        LAST_PROF.append(resB)

    e_atom = np.zeros(N_ATOMS, np.float32)
    for core in range(NCORES):
        amap = tabs[core]["amap"].reshape(2, KPC)
        oute = resB.results[core]["oute"]  # [2, KPC]
        for b in range(2):
            valid = amap[b] >= 0
            e_atom[amap[b][valid]] = oute[b, valid]
    e_atom += comp_weights[species]
    out = np.zeros(N_STRUCT, np.float32)
    np.add.at(out, structure_ids_np, e_atom)
    return out
